# revision 1
# baseline (speedup 1.0000x reference)
# GRU decoder kernel for Trainium2 (Bass/Tile), data-parallel over batch.
#
# Problem (per reference):
#   h0 = tanh(latent @ Wd + bd)                      [B, H]
#   x  = latent @ W + b[0]; xz, xr, xh = split(x, 3) [B, 3H]
#   for t in range(T):   (reset_after GRU, recurrent bias b[1])
#       rec = h @ U + b[1]; rz, rr, rh = split(rec, 3)
#       z = sigmoid(xz + rz); r = sigmoid(xr + rr)
#       hh = tanh(xh + r * rh)
#       h = z*h + (1-z)*hh        -> out[:, t, :]
#
# Sharding: batch 1024 -> 8 cores x 128 rows. Weights replicated. The T loop
# runs locally per core; no collectives.
#
# Per-core per-step dataflow (layout [batch=partitions, features=free]):
#   PE   : per gate g in (r, h, z): identity-matmul accumulates the x-side
#          bias/projection into that gate's own PSUM bank, then 4 K-chunk
#          matmuls of h_T @ U.  float32r operands (1 cycle/row vs 4 for f32).
#          z is issued last: it is consumed late, so its matmuls fill PE idle
#          time during the tail.
#   ACT  : r = sigmoid(ps_r), z = sigmoid(ps_z), zc = sigmoid(-ps_z) [= 1-z],
#          hh = tanh(t2), half the hT copies.
#   DVE  : t1 = r*ps_h, t2 = t1+xh, d = zc*hh, h_new = c1+d (4x128 chunks),
#          half the hT copies.
#   GP   : c1 = z*h
#   PE   : per-128-chunk transpose h_new -> own PSUM bank -> SBUF hT_j copy.
#   DMA  : h_new -> out[:, t, :]
#
# Separate tiles per gate-PSUM / per hT chunk keep Tile's dependency tracking
# fine-grained (a reader only waits for its own producer, not the whole
# 15-matmul burst).

import numpy as np

B, LD, H, T_DEF = 1024, 256, 512, 128
H3 = 3 * H
NCORES = 8
BS = B // NCORES  # 128 batch rows per core

_BUILD_CACHE = {}


def _build(T, tail_chunks=2):
    import concourse.bass as bass
    import concourse.mybir as mybir
    import concourse.tile as tile
    from concourse import bacc
    from concourse.masks import make_identity

    f32 = mybir.dt.float32
    f32r = mybir.dt.float32r
    AF = mybir.ActivationFunctionType
    OP = mybir.AluOpType

    nc = bacc.Bacc(None, target_bir_lowering=False, debug=False)

    latT = nc.dram_tensor("latT", [LD, BS], f32, kind="ExternalInput")
    wd_d = nc.dram_tensor("wd", [LD, H], f32, kind="ExternalInput")
    w_d = nc.dram_tensor("w", [LD, H3], f32, kind="ExternalInput")
    u_d = nc.dram_tensor("u", [H, H3], f32r, kind="ExternalInput")
    # bx = b[0] with b[1] folded into the z/r thirds; bh = b[1] h-third
    bx_d = nc.dram_tensor("bx", [H3], f32, kind="ExternalInput")
    bh_d = nc.dram_tensor("bh", [H], f32r, kind="ExternalInput")
    bd_d = nc.dram_tensor("bd", [H], f32, kind="ExternalInput")
    out_d = nc.dram_tensor("out", [BS, T, H], f32, kind="ExternalOutput")

    # gate column ranges in the 3H axis (reference order: z, r, h)
    ZS, RS, HS = slice(0, H), slice(H, 2 * H), slice(2 * H, H3)

    with tile.TileContext(nc) as tc:
        with (
            tc.tile_pool(name="singles", bufs=1) as singles,
            tc.tile_pool(name="work", bufs=3) as work,
            tc.tile_pool(name="hpool", bufs=3) as hpool,
            tc.tile_pool(name="htpool", bufs=3) as htpool,
            tc.tile_pool(name="psg", bufs=1, space="PSUM") as psg,
            tc.tile_pool(name="pst", bufs=1, space="PSUM") as pst,
        ):
            # ---- load constants -------------------------------------------
            lat = [singles.tile([128, BS], f32, tag=f"lat{j}", name=f"lat{j}")
                   for j in range(2)]
            for j in range(2):
                nc.sync.dma_start(out=lat[j], in_=latT[128 * j : 128 * (j + 1), :])
            wd = [singles.tile([128, H], f32, tag=f"wd{j}", name=f"wd{j}")
                  for j in range(2)]
            for j in range(2):
                nc.sync.dma_start(out=wd[j], in_=wd_d[128 * j : 128 * (j + 1), :])
            w = [singles.tile([128, H3], f32, tag=f"w{j}", name=f"w{j}")
                 for j in range(2)]
            for j in range(2):
                nc.sync.dma_start(out=w[j], in_=w_d[128 * j : 128 * (j + 1), :])
            u = [singles.tile([128, H3], f32r, tag=f"u{k}", name=f"u{k}")
                 for k in range(4)]
            for k in range(4):
                nc.sync.dma_start(out=u[k], in_=u_d[128 * k : 128 * (k + 1), :])

            def bcast(handle, n):
                ap = handle[:]
                return bass.AP(tensor=ap.tensor, offset=ap.offset,
                               ap=[[0, 128], [1, n]])

            xbias = singles.tile([128, H3], f32, tag="xbias")
            nc.gpsimd.dma_start(out=xbias, in_=bcast(bx_d, H3))
            b1h = singles.tile([128, H], f32r, tag="b1h")
            nc.gpsimd.dma_start(out=b1h, in_=bcast(bh_d, H))
            bdt = singles.tile([128, H], f32, tag="bdt")
            nc.gpsimd.dma_start(out=bdt, in_=bcast(bd_d, H))

            ident = singles.tile([128, 128], f32, tag="ident")
            make_identity(nc, ident)
            identr = singles.tile([128, 128], f32r, tag="identr")
            nc.scalar.copy(identr, ident)

            # PSUM tiles: 3 gate banks + 4 transpose banks + 1 prologue = 8
            ps_z = psg.tile([128, H], f32, tag="ps_z")
            ps_r = psg.tile([128, H], f32, tag="ps_r")
            ps_h = psg.tile([128, H], f32, tag="ps_h")
            gate_ps = [ps_z, ps_r, ps_h]
            tp = [pst.tile([128, 128], f32, tag=f"tp{j}", name=f"tp{j}")
                  for j in range(4)]
            pd = pst.tile([128, H], f32, tag="pd")

            # ---- prologue: h0 and x-projection (full fp32 precision) ------
            nc.tensor.matmul(pd, ident, bdt, start=True, stop=False)
            nc.tensor.matmul(pd, lat[0], wd[0], start=False, stop=False)
            nc.tensor.matmul(pd, lat[1], wd[1], start=False, stop=True)
            h = hpool.tile([128, H], f32, tag="h")
            nc.scalar.activation(h, pd, AF.Tanh)

            for gi, s in ((0, ZS), (1, RS), (2, HS)):
                px = gate_ps[gi]
                nc.tensor.matmul(px, ident, xbias[:, s], start=True, stop=False)
                nc.tensor.matmul(px, lat[0], w[0][:, s], start=False, stop=False)
                nc.tensor.matmul(px, lat[1], w[1][:, s], start=False, stop=True)
            # x-projection: rounded f32r copy for matmul use + fp32 copy of xh
            xzr = singles.tile([128, 2 * H], f32r, tag="xzr")
            nc.scalar.copy(xzr[:, ZS], gate_ps[0])
            nc.scalar.copy(xzr[:, RS], gate_ps[1])
            xh32 = singles.tile([128, H], f32, tag="xh32")
            nc.scalar.copy(xh32, gate_ps[2])

            hT = [htpool.tile([128, 128], f32r, tag=f"hT{j}", name=f"hT{j}")
                  for j in range(4)]
            for j in range(4):
                cs = slice(128 * j, 128 * (j + 1))
                nc.tensor.transpose(tp[j], h[:, cs], ident)
                nc.scalar.copy(hT[j], tp[j])

            # ---- steady-state T loop --------------------------------------
            nch = H // tail_chunks
            for t in range(T):
                # gate matmul bursts; r first (needed earliest), h last
                ps_z = psg.tile([128, H], f32, tag="ps_z")
                ps_r = psg.tile([128, H], f32, tag="ps_r")
                ps_h = psg.tile([128, H], f32, tag="ps_h")
                # r and h bursts gate the tail; z matmuls are issued after
                # (they run in PE idle time during the tail -- z is only
                # consumed by zc/c1/d, late in the step)
                for ps, s, xsrc in ((ps_r, RS, xzr[:, RS]), (ps_h, HS, b1h),
                                    (ps_z, ZS, xzr[:, ZS])):
                    nc.tensor.matmul(ps, identr, xsrc, start=True, stop=False)
                    for k in range(4):
                        nc.tensor.matmul(ps, hT[k], u[k][:, s],
                                         start=False, stop=(k == 3))

                r = work.tile([128, H], f32, tag="r")
                nc.scalar.activation(r, ps_r, AF.Sigmoid)
                z = work.tile([128, H], f32, tag="z")
                nc.scalar.activation(z, ps_z, AF.Sigmoid)
                # zc = 1-z via sigmoid(-pre_z) on ACT; c1 = z*h on GPSIMD --
                # both off the DVE critical chain
                zc = work.tile([128, H], f32, tag="zc")
                nc.scalar.activation(zc, ps_z, AF.Sigmoid, scale=-1.0)
                c1 = work.tile([128, H], f32, tag="c1")
                for lo in range(0, H, 128):
                    nc.gpsimd.tensor_mul(c1[:, lo:lo+128], z[:, lo:lo+128],
                                         h[:, lo:lo+128])

                # h_new = c1 + zc*hh, chunked so chunk 0's hT copies unblock
                # the next burst's k=0/1 matmuls early
                chunks = [(0, 128), (128, 256), (256, 384), (384, H)]
                t1 = work.tile([128, H], f32, tag="t1")
                t2 = work.tile([128, H], f32, tag="t2")
                hh = work.tile([128, H], f32, tag="hh")
                d = work.tile([128, H], f32, tag="d")
                hnew = hpool.tile([128, H], f32, tag="h")
                hT_new = [htpool.tile([128, 128], f32r, tag=f"hT{j}",
                                      name=f"hTn{j}") for j in range(4)]
                tpn = [pst.tile([128, 128], f32, tag=f"tp{j}", name=f"tpn{j}")
                       for j in range(4)]
                for lo, hi in chunks:
                    cs = slice(lo, hi)
                    nc.vector.tensor_mul(t1[:, cs], r[:, cs], ps_h[:, cs])
                    nc.vector.tensor_add(t2[:, cs], t1[:, cs], xh32[:, cs])
                    nc.scalar.activation(hh[:, cs], t2[:, cs], AF.Tanh)
                    nc.vector.tensor_mul(d[:, cs], zc[:, cs], hh[:, cs])
                    nc.vector.tensor_add(hnew[:, cs], c1[:, cs], d[:, cs])
                    for j in range(lo // 128, hi // 128):
                        js = slice(128 * j, 128 * (j + 1))
                        nc.tensor.transpose(tpn[j], hnew[:, js], ident)
                        # alternate copy engine so the 4 copies pairwise overlap
                        if j % 2 == 0:
                            nc.scalar.copy(hT_new[j], tpn[j])
                        else:
                            nc.vector.tensor_copy(hT_new[j], tpn[j])

                nc.sync.dma_start(out=out_d[:, t, :], in_=hnew)
                h = hnew
                hT = hT_new

    nc.compile()
    return nc


def kernel(latent, Wd, bd, W, U, b, T, _trace=False):
    from concourse.bass_utils import run_bass_kernel_spmd

    latent = np.ascontiguousarray(np.asarray(latent, dtype=np.float32))
    Wd = np.ascontiguousarray(np.asarray(Wd, dtype=np.float32))
    bd = np.ascontiguousarray(np.asarray(bd, dtype=np.float32))
    W = np.ascontiguousarray(np.asarray(W, dtype=np.float32))
    U = np.ascontiguousarray(np.asarray(U, dtype=np.float32))
    b = np.ascontiguousarray(np.asarray(b, dtype=np.float32))
    T = int(T)

    key = (T,)
    if key not in _BUILD_CACHE:
        _BUILD_CACHE[key] = _build(T)
    nc = _BUILD_CACHE[key]

    bx = b[0].copy()
    bx[: 2 * H] += b[1][: 2 * H]
    bh = np.ascontiguousarray(b[1][2 * H :])

    in_maps = []
    for c in range(NCORES):
        rows = slice(c * BS, (c + 1) * BS)
        in_maps.append({
            "latT": np.ascontiguousarray(latent[rows].T),
            "wd": Wd, "w": W, "u": U,
            "bx": bx, "bh": bh, "bd": bd,
        })

    res = run_bass_kernel_spmd(nc, in_maps, core_ids=list(range(NCORES)),
                               trace=_trace)
    if _trace and res.exec_time_ns is not None:
        print(f"HW exec time: {res.exec_time_ns} ns")
        if res.instructions_and_trace is not None:
            print(f"trace: {res.instructions_and_trace[1]}")

    out = np.concatenate([r["out"] for r in res.results], axis=0)
    return out



# revision 8
# speedup vs baseline: 1.2443x; 1.2443x over previous
# GRU decoder kernel for Trainium2 (Bass/Tile), data-parallel over batch.
#
# Problem (per reference):
#   h0 = tanh(latent @ Wd + bd)                      [B, H]
#   x  = latent @ W + b[0]; xz, xr, xh = split(x, 3) [B, 3H]
#   for t in range(T):   (reset_after GRU, recurrent bias b[1])
#       rec = h @ U + b[1]; rz, rr, rh = split(rec, 3)
#       z = sigmoid(xz + rz); r = sigmoid(xr + rr)
#       hh = tanh(xh + r * rh)
#       h = z*h + (1-z)*hh        -> out[:, t, :]
#
# Sharding: batch 1024 -> 8 cores x 128 rows, weights replicated, T loop
# local per core (no collectives).
#
# Layout: everything lives TRANSPOSED and "chunk-stacked": a [B, H] tensor
# is stored as a [128, 512] tile S with S[p, 128*m + b] = X[b, 128*m + p]
# (m = H-chunk, p = row within chunk, b = batch row). In this layout the
# hidden state IS the matmul moving operand (no transposes, no hT copies):
#   ps_g[:, 128m:128(m+1)] += U[128k:128(k+1), 512g+128m:...]^T @ h[:, 128k:...]
#
# Delta-accumulation: the three gate PSUM banks persistently hold
#   ps_z = xz + b1z + h_t@Uz,  ps_r = xr + b1r + h_t@Ur,  ps_h = b1h + h_t@Uh
# initialized once in the prologue; each step accumulates only
# dl = h_{t+1} - h_t through U (start=False), so there is no per-step
# bias/x-projection cost on any engine.  Matmul groups fire per dl chunk
# (k-chunks of that tail chunk) the moment the chunk is ready, ordered
# r-gate first (spine head), then z (zb is needed mid-tail), then h.
#
# Step tail per chunk (fp16):
#   r  = sigmoid(ps_r)            [ACT]
#   zb = sigmoid(-ps_z) (= 1-z)   [ACT]
#   t1 = r * ps_h                 [DVE]
#   t2 = t1 + xh                  [DVE]
#   hh = tanh(t2)                 [ACT]
#   e  = hh - h                   [DVE]
#   dl = zb * e   (= h' - h)      [DVE] -> feeds next matmul round
#   h' = h + dl                   [Pool] -> DMA out (fp16, host converts)

import numpy as np

B, LD, H, T_DEF = 1024, 256, 512, 128
H3 = 3 * H
NCORES = 8
BS = B // NCORES  # 128 batch rows per core

_BUILD_CACHE = {}
_LABELS = {}


def _lab(r, s):
    try:
        _LABELS[r.ins.name] = s
    except Exception:
        pass
    return r

# tail chunk boundaries in the stacked free dim (multiples of 128)
CFG = dict(
    chunks=(0, 256, 512),    # matmul-group split (dl chunks that fire groups)
    sr_chunks=(0, 256, 512),  # sigmoid(r) op split
    zb_chunks=(0, 256, 512),  # zb op split
    t_chunks=(0, 256, 512),   # t1/t2 op split
    th_chunks=(0, 256, 512),  # tanh op split
    e_chunks=(0, 256, 512),   # e op split
    dl_chunks=(0, 256, 512),  # dl op split (>= mm chunk granularity)
    gate_order="rzh",        # gate order inside each matmul group
    mm_blocks="GM",          # explicit (gate, mset, kset) block list
)


# "smart" block order for the (0,256,512) split: r both halves first
# (spine head), then h/z interleaved by m-subset in due-time order.
MM_SMART = [
    ("r", (0, 1, 2, 3), (0, 1)),
    ("r", (0, 1), (2, 3)), ("r", (2, 3), (2, 3)),
    ("h", (0, 1, 2, 3), (0, 1)),
    ("h", (0, 1), (2, 3)),
    ("z", (0, 1), (0, 1)), ("z", (0, 1), (2, 3)),
    ("h", (2, 3), (2, 3)),
    ("z", (2, 3), (0, 1)), ("z", (2, 3), (2, 3)),
]

# gate-major: r both halves, h both, z both (in-order PE friendly)
MM_GM = [
    ("r", (0, 1, 2, 3), (0, 1)),
    ("r", (0, 1), (2, 3)), ("r", (2, 3), (2, 3)),
    ("h", (0, 1, 2, 3), (0, 1)),
    ("h", (0, 1), (2, 3)), ("h", (2, 3), (2, 3)),
    ("z", (0, 1, 2, 3), (0, 1)),
    ("z", (0, 1), (2, 3)), ("z", (2, 3), (2, 3)),
]


def _build(T, cfg=CFG):
    import concourse.bass as bass
    import concourse.mybir as mybir
    import concourse.tile as tile
    from concourse import bacc

    f32 = mybir.dt.float32
    f16 = mybir.dt.float16
    AF = mybir.ActivationFunctionType
    OP = mybir.AluOpType

    nc = bacc.Bacc(None, target_bir_lowering=False, debug=False)

    latT_d = nc.dram_tensor("latT", [LD, BS], f16, kind="ExternalInput")
    wd_d = nc.dram_tensor("wd", [LD, H], f16, kind="ExternalInput")
    w_d = nc.dram_tensor("w", [LD, H3], f16, kind="ExternalInput")
    u_d = nc.dram_tensor("u", [H, H3], f16, kind="ExternalInput")
    # bx = b[0] with b[1] z/r parts folded in; b1h = b[1] h third; bd
    bx_d = nc.dram_tensor("bx", [1, H3], f16, kind="ExternalInput")
    b1h_d = nc.dram_tensor("b1h", [1, H], f16, kind="ExternalInput")
    bd_d = nc.dram_tensor("bd", [1, H], f16, kind="ExternalInput")
    out_d = nc.dram_tensor("out", [T, 128, H], f16, kind="ExternalOutput")

    nk = H // 128    # 4 k-chunks over hidden
    nm = H // 128    # 4 m-chunks per gate
    nkl = LD // 128  # 2 k-chunks over latent
    # gate order in U/W columns (reference): z=0, r=1, h=2
    GZ, GR, GH = 0, 1, 2
    GMAP = {"r": GR, "z": GZ, "h": GH}
    gate_seq = [GMAP[ch] for ch in cfg["gate_order"]]
    spans = list(zip(cfg["chunks"][:-1], cfg["chunks"][1:]))

    def _spans(key):
        c = cfg.get(key, cfg["chunks"])
        return list(zip(c[:-1], c[1:]))

    sr_spans = _spans("sr_chunks")
    zspans = _spans("zb_chunks")
    t_spans = _spans("t_chunks")
    th_spans = _spans("th_chunks")
    e_spans = _spans("e_chunks")
    dl_spans = _spans("dl_chunks")

    with tile.TileContext(nc) as tc:
        with (
            tc.tile_pool(name="singles", bufs=1) as singles,
            tc.tile_pool(name="hpool", bufs=3) as hpool,
            tc.tile_pool(name="dpool", bufs=3) as dpool,
            tc.tile_pool(name="work", bufs=2) as work,
            tc.tile_pool(name="psg", bufs=1, space="PSUM") as psg,
        ):
            # ---- load constants -------------------------------------------
            lat = [singles.tile([128, BS], f16, tag=f"lat{j}", name=f"lat{j}")
                   for j in range(nkl)]
            for j in range(nkl):
                nc.sync.dma_start(out=lat[j], in_=latT_d[128 * j:128 * (j + 1), :])
            wd = [singles.tile([128, H], f16, tag=f"wd{j}", name=f"wd{j}")
                  for j in range(nkl)]
            for j in range(nkl):
                nc.sync.dma_start(out=wd[j], in_=wd_d[128 * j:128 * (j + 1), :])
            w = [singles.tile([128, H3], f16, tag=f"w{j}", name=f"w{j}")
                 for j in range(nkl)]
            for j in range(nkl):
                nc.sync.dma_start(out=w[j], in_=w_d[128 * j:128 * (j + 1), :])
            u = [singles.tile([128, H3], f16, tag=f"u{k}", name=f"u{k}")
                 for k in range(nk)]
            for k in range(nk):
                nc.sync.dma_start(out=u[k], in_=u_d[128 * k:128 * (k + 1), :])
            bx = singles.tile([1, H3], f16, tag="bx")
            nc.gpsimd.dma_start(out=bx, in_=bx_d[:, :])
            b1h = singles.tile([1, H], f16, tag="b1h")
            nc.gpsimd.dma_start(out=b1h, in_=b1h_d[:, :])
            bd = singles.tile([1, H], f16, tag="bd")
            nc.gpsimd.dma_start(out=bd, in_=bd_d[:, :])
            ones = singles.tile([1, 128], f16, tag="ones")
            nc.vector.memset(ones, 1.0)

            # persistent gate banks + 2 prologue scratch banks
            ps_z = psg.tile([128, H], f32, tag="ps_z")
            ps_r = psg.tile([128, H], f32, tag="ps_r")
            ps_h = psg.tile([128, H], f32, tag="ps_h")
            ps_a = psg.tile([128, H], f32, tag="ps_a")
            ps_b = psg.tile([128, H], f32, tag="ps_b")
            gate_ps = {GZ: ps_z, GR: ps_r, GH: ps_h}

            def cs(m):
                return slice(128 * m, 128 * (m + 1))

            # ---- prologue --------------------------------------------------
            # PSUM start_tensor_calc lazily zeroes the WHOLE 2KB zero region
            # (= the full bank row), so each bank gets exactly ONE start=True
            # (its first matmul); every other matmul accumulates.  The first
            # write to each not-yet-touched region then replaces (pending
            # zero), later writes accumulate -- which is what we want.
            # h0 = tanh((latent @ Wd)^T + bd), stacked
            for m in range(nm):
                for j in range(nkl):
                    nc.tensor.matmul(ps_a[:, cs(m)], wd[j][:, cs(m)], lat[j],
                                     start=(j == 0 and m == 0), stop=False,
                                     skip_group_check=True)
                nc.tensor.matmul(ps_a[:, cs(m)], bd[:, cs(m)], ones,
                                 start=False, stop=True, skip_group_check=True)
            h = hpool.tile([128, H], f16, tag="h")
            nc.scalar.activation(h, ps_a, AF.Tanh)

            # xh = (latent @ W_h)^T + bx_h, stacked, fp16 in SBUF
            for m in range(nm):
                for j in range(nkl):
                    nc.tensor.matmul(ps_b[:, cs(m)], w[j][:, GH * H + 128 * m:
                                                          GH * H + 128 * (m + 1)],
                                     lat[j], start=(j == 0 and m == 0),
                                     stop=False, skip_group_check=True)
                nc.tensor.matmul(ps_b[:, cs(m)],
                                 bx[:, GH * H + 128 * m:GH * H + 128 * (m + 1)],
                                 ones, start=False, stop=True,
                                 skip_group_check=True)
            xh = singles.tile([128, H], f16, tag="xh")
            nc.scalar.copy(xh, ps_b)

            # gate banks: x-projection + bias + h0 @ U_g
            for g in (GZ, GR, GH):
                ps = gate_ps[g]
                first = [True]

                def mm(dst, lhsT, rhs, stop=False):
                    nc.tensor.matmul(dst, lhsT, rhs, start=first[0], stop=stop,
                                     skip_group_check=True)
                    first[0] = False

                for m in range(nm):
                    if g == GH:
                        # h gate: recurrent bias only (xh is separate)
                        mm(ps[:, cs(m)], b1h[:, cs(m)], ones)
                    else:
                        for j in range(nkl):
                            mm(ps[:, cs(m)],
                               w[j][:, g * H + 128 * m:g * H + 128 * (m + 1)],
                               lat[j])
                        mm(ps[:, cs(m)],
                           bx[:, g * H + 128 * m:g * H + 128 * (m + 1)],
                           ones)
                    for k in range(nk):
                        mm(ps[:, cs(m)],
                           u[k][:, g * H + 128 * m:g * H + 128 * (m + 1)],
                           h[:, cs(k)], stop=(k == nk - 1))

            # ---- steady-state T loop --------------------------------------
            dl_prev = None
            for t in range(T):
                # matmul round t: ps_g += dl_{t} @ U_g, one group per dl
                # chunk, fired as soon as that chunk exists.  Gate order:
                # r first (spine head), then z (zb needed mid-tail), then h.
                if dl_prev is not None:
                    mmb = cfg.get("mm_blocks")
                    if mmb == "GM":
                        mmb = MM_GM
                    elif mmb == "SMART":
                        mmb = MM_SMART
                    if mmb:
                        blocks = [(GMAP[gc], ms, ks)
                                  for gc, ms, ks in mmb]
                    else:
                        blocks = []
                        for (lo, hi) in spans:
                            ks = tuple(range(lo // 128, hi // 128))
                            for g in gate_seq:
                                blocks.append((g, tuple(range(nm)), ks))
                    for g, ms, ks in blocks:
                        ps = gate_ps[g]
                        for m in ms:
                            for k in ks:
                                _lab(nc.tensor.matmul(
                                    ps[:, cs(m)],
                                    u[k][:, g * H + 128 * m:
                                         g * H + 128 * (m + 1)],
                                    dl_prev[:, cs(k)],
                                    start=False, stop=(k == nk - 1),
                                    skip_group_check=True),
                                    f"mm.{'zrh'[g]}.m{m}.k{k}")

                # tail t
                r = work.tile([128, H], f16, tag="r")
                zb = work.tile([128, H], f16, tag="zb")
                t1 = work.tile([128, H], f16, tag="t1")
                t2 = work.tile([128, H], f16, tag="t2")
                hh = work.tile([128, H], f16, tag="hh")
                e = work.tile([128, H], f16, tag="e")
                dl = dpool.tile([128, H], f16, tag="dl", name=f"dl{t % 3}")
                hn = hpool.tile([128, H], f16, tag="h", name=f"h{t % 3}")

                for ci, (lo, hi) in enumerate(sr_spans):
                    sp = slice(lo, hi)
                    _lab(nc.scalar.activation(r[:, sp], ps_r[:, sp],
                                              AF.Sigmoid), f"sr.c{ci}")
                for ci, (lo, hi) in enumerate(zspans):
                    sp = slice(lo, hi)
                    _lab(nc.scalar.activation(zb[:, sp], ps_z[:, sp],
                                              AF.Sigmoid, scale=-1.0),
                         f"zb.c{ci}")
                for ci, (lo, hi) in enumerate(t_spans):
                    sp = slice(lo, hi)
                    _lab(nc.vector.tensor_tensor(out=t1[:, sp], in0=r[:, sp],
                                                 in1=ps_h[:, sp], op=OP.mult),
                         f"t1.c{ci}")
                    _lab(nc.vector.tensor_tensor(out=t2[:, sp], in0=t1[:, sp],
                                                 in1=xh[:, sp], op=OP.add),
                         f"t2.c{ci}")
                for ci, (lo, hi) in enumerate(th_spans):
                    sp = slice(lo, hi)
                    _lab(nc.scalar.activation(hh[:, sp], t2[:, sp], AF.Tanh),
                         f"th.c{ci}")
                for ci, (lo, hi) in enumerate(e_spans):
                    sp = slice(lo, hi)
                    _lab(nc.vector.tensor_tensor(out=e[:, sp], in0=hh[:, sp],
                                                 in1=h[:, sp],
                                                 op=OP.subtract), f"e.c{ci}")
                for ci, (lo, hi) in enumerate(dl_spans):
                    sp = slice(lo, hi)
                    _lab(nc.vector.tensor_tensor(out=dl[:, sp], in0=zb[:, sp],
                                                 in1=e[:, sp], op=OP.mult),
                         f"dl.c{ci}")
                for ci, (lo, hi) in enumerate(spans):
                    sp = slice(lo, hi)
                    _lab(nc.gpsimd.tensor_tensor(out=hn[:, sp], in0=h[:, sp],
                                                 in1=dl[:, sp], op=OP.add),
                         f"hn.c{ci}")

                _lab(nc.sync.dma_start(out=out_d[t], in_=hn), "dma.out")
                dl_prev = dl
                h = hn

    nc.compile()
    return nc


def kernel(latent, Wd, bd, W, U, b, T, _trace=False):
    from concourse.bass_utils import run_bass_kernel_spmd

    latent = np.asarray(latent, dtype=np.float32)
    Wd = np.asarray(Wd, dtype=np.float32)
    bd = np.asarray(bd, dtype=np.float32)
    W = np.asarray(W, dtype=np.float32)
    U = np.asarray(U, dtype=np.float32)
    b = np.asarray(b, dtype=np.float32)
    T = int(T)

    key = (T,)
    if key not in _BUILD_CACHE:
        _BUILD_CACHE[key] = _build(T)
    nc = _BUILD_CACHE[key]

    bx = b[0].copy()
    bx[: 2 * H] += b[1][: 2 * H]
    bx16 = np.ascontiguousarray(bx.reshape(1, H3)).astype(np.float16)
    b1h16 = np.ascontiguousarray(b[1][2 * H:].reshape(1, H)).astype(np.float16)
    bd16 = np.ascontiguousarray(bd.reshape(1, H)).astype(np.float16)
    u16 = np.ascontiguousarray(U).astype(np.float16)
    w16 = np.ascontiguousarray(W).astype(np.float16)
    wd16 = np.ascontiguousarray(Wd).astype(np.float16)

    in_maps = []
    for c in range(NCORES):
        rows = slice(c * BS, (c + 1) * BS)
        in_maps.append({
            "latT": np.ascontiguousarray(latent[rows].T).astype(np.float16),
            "wd": wd16, "w": w16, "u": u16,
            "bx": bx16, "b1h": b1h16, "bd": bd16,
        })

    res = run_bass_kernel_spmd(nc, in_maps, core_ids=list(range(NCORES)),
                               trace=_trace)
    if _trace and res.exec_time_ns is not None:
        print(f"HW exec time: {res.exec_time_ns} ns")
        if res.instructions_and_trace is not None:
            print(f"trace: {res.instructions_and_trace[1]}")

    # device output is [T, 128, 4*128] stacked-transposed fp16:
    #   arr[t, p, 128*m + b] = h_{t+1}[b, 128*m + p]
    outs = []
    for c in range(NCORES):
        arr = res.results[c]["out"]  # [T, 128, 512] fp16
        arr = arr.reshape(T, 128, H // 128, 128).transpose(3, 0, 2, 1)
        outs.append(arr.reshape(BS, T, H))
    return np.ascontiguousarray(np.concatenate(outs, axis=0)).astype(np.float32)


# revision 14
# speedup vs baseline: 1.5374x; 1.2355x over previous
# GRU decoder kernel for Trainium2 (Bass/Tile), data-parallel over batch.
#
# Problem (per reference):
#   h0 = tanh(latent @ Wd + bd)                      [B, H]
#   x  = latent @ W + b[0]; xz, xr, xh = split(x, 3) [B, 3H]
#   for t in range(T):   (reset_after GRU, recurrent bias b[1])
#       rec = h @ U + b[1]; rz, rr, rh = split(rec, 3)
#       z = sigmoid(xz + rz); r = sigmoid(xr + rr)
#       hh = tanh(xh + r * rh)
#       h = z*h + (1-z)*hh        -> out[:, t, :]
#
# Sharding: batch 1024 -> 8 cores x 128 rows, weights replicated, T loop
# local per core (no collectives).
#
# Layout: everything lives TRANSPOSED and "chunk-stacked": a [B, H] tensor
# is stored as a [128, 512] tile S with S[p, 128*m + b] = X[b, 128*m + p]
# (m = H-chunk, p = row within chunk, b = batch row). In this layout the
# hidden state IS the matmul moving operand (no transposes, no hT copies):
#   ps_g[:, 128m:128(m+1)] += U[128k:128(k+1), 512g+128m:...]^T @ h[:, 128k:...]
#
# Delta-accumulation: the three gate PSUM banks persistently hold
#   ps_z = xz + b1z + h_t@Uz,  ps_r = xr + b1r + h_t@Ur,  ps_h = b1h + h_t@Uh
# initialized once in the prologue; each step accumulates only
# dl = h_{t+1} - h_t through U (start=False), so there is no per-step
# bias/x-projection cost on any engine.  Matmul groups fire per dl chunk
# (k-chunks of that tail chunk) the moment the chunk is ready, ordered
# r-gate first (spine head), then z (zb is needed mid-tail), then h.
#
# Step tail per chunk (fp16):
#   r  = sigmoid(ps_r)            [ACT]
#   zb = sigmoid(-ps_z) (= 1-z)   [ACT]
#   t1 = r * ps_h                 [DVE]
#   t2 = t1 + xh                  [DVE]
#   hh = tanh(t2)                 [ACT]
#   e  = hh - h                   [DVE]
#   dl = zb * e   (= h' - h)      [DVE] -> feeds next matmul round
#   h' = h + dl                   [Pool] -> DMA out (fp16, host converts)

import numpy as np

B, LD, H, T_DEF = 1024, 256, 512, 128
H3 = 3 * H
NCORES = 8
BS = B // NCORES  # 128 batch rows per core

_BUILD_CACHE = {}
_LABELS = {}
ARCH = "dual"  # "dual" (2 independent 64-batch chains) or "single"


def _lab(r, s):
    try:
        _LABELS[r.ins.name] = s
    except Exception:
        pass
    return r

# tail chunk boundaries in the stacked free dim (multiples of 128)
CFG = dict(
    chunks=(0, 256, 512),    # matmul-group split (dl chunks that fire groups)
    sr_chunks=(0, 256, 512),  # sigmoid(r) op split
    zb_chunks=(0, 256, 512),  # zb op split
    t_chunks=(0, 256, 512),   # t1/t2 op split
    th_chunks=(0, 256, 512),  # tanh op split
    e_chunks=(0, 256, 512),   # e op split
    dl_chunks=(0, 256, 512),  # dl op split (>= mm chunk granularity)
    gate_order="rzh",        # gate order inside each matmul group
    mm_blocks="GM",          # explicit (gate, mset, kset) block list
)


# "smart" block order for the (0,256,512) split: r both halves first
# (spine head), then h/z interleaved by m-subset in due-time order.
MM_SMART = [
    ("r", (0, 1, 2, 3), (0, 1)),
    ("r", (0, 1), (2, 3)), ("r", (2, 3), (2, 3)),
    ("h", (0, 1, 2, 3), (0, 1)),
    ("h", (0, 1), (2, 3)),
    ("z", (0, 1), (0, 1)), ("z", (0, 1), (2, 3)),
    ("h", (2, 3), (2, 3)),
    ("z", (2, 3), (0, 1)), ("z", (2, 3), (2, 3)),
]

# gate-major: r both halves, h both, z both (in-order PE friendly)
MM_GM = [
    ("r", (0, 1, 2, 3), (0, 1)),
    ("r", (0, 1), (2, 3)), ("r", (2, 3), (2, 3)),
    ("h", (0, 1, 2, 3), (0, 1)),
    ("h", (0, 1), (2, 3)), ("h", (2, 3), (2, 3)),
    ("z", (0, 1, 2, 3), (0, 1)),
    ("z", (0, 1), (2, 3)), ("z", (2, 3), (2, 3)),
]

# taper (0,384,512): k012 groups fired by the big chunk, k3 by the small
MM_GM_TAPER = [
    ("r", (0, 1, 2, 3), (0, 1, 2)),
    ("r", (0, 1), (3,)), ("r", (2, 3), (3,)),
    ("h", (0, 1, 2, 3), (0, 1, 2)),
    ("h", (0, 1), (3,)), ("h", (2, 3), (3,)),
    ("z", (0, 1, 2, 3), (0, 1, 2)),
    ("z", (0, 1), (3,)), ("z", (2, 3), (3,)),
]


def _build(T, cfg=CFG):
    import concourse.bass as bass
    import concourse.mybir as mybir
    import concourse.tile as tile
    from concourse import bacc

    f32 = mybir.dt.float32
    f16 = mybir.dt.float16
    AF = mybir.ActivationFunctionType
    OP = mybir.AluOpType

    nc = bacc.Bacc(None, target_bir_lowering=False, debug=False)

    latT_d = nc.dram_tensor("latT", [LD, BS], f16, kind="ExternalInput")
    wd_d = nc.dram_tensor("wd", [LD, H], f16, kind="ExternalInput")
    w_d = nc.dram_tensor("w", [LD, H3], f16, kind="ExternalInput")
    u_d = nc.dram_tensor("u", [H, H3], f16, kind="ExternalInput")
    # bx = b[0] with b[1] z/r parts folded in; b1h = b[1] h third; bd
    bx_d = nc.dram_tensor("bx", [1, H3], f16, kind="ExternalInput")
    b1h_d = nc.dram_tensor("b1h", [1, H], f16, kind="ExternalInput")
    bd_d = nc.dram_tensor("bd", [1, H], f16, kind="ExternalInput")
    out_d = nc.dram_tensor("out", [T, 128, H], f16, kind="ExternalOutput")

    nk = H // 128    # 4 k-chunks over hidden
    nm = H // 128    # 4 m-chunks per gate
    nkl = LD // 128  # 2 k-chunks over latent
    # gate order in U/W columns (reference): z=0, r=1, h=2
    GZ, GR, GH = 0, 1, 2
    GMAP = {"r": GR, "z": GZ, "h": GH}
    gate_seq = [GMAP[ch] for ch in cfg["gate_order"]]
    spans = list(zip(cfg["chunks"][:-1], cfg["chunks"][1:]))

    def _spans(key):
        c = cfg.get(key, cfg["chunks"])
        return list(zip(c[:-1], c[1:]))

    sr_spans = _spans("sr_chunks")
    zspans = _spans("zb_chunks")
    t_spans = _spans("t_chunks")
    th_spans = _spans("th_chunks")
    e_spans = _spans("e_chunks")
    dl_spans = _spans("dl_chunks")

    with tile.TileContext(nc) as tc:
        with (
            tc.tile_pool(name="singles", bufs=1) as singles,
            tc.tile_pool(name="hpool", bufs=3) as hpool,
            tc.tile_pool(name="dpool", bufs=3) as dpool,
            tc.tile_pool(name="work", bufs=3) as work,
            tc.tile_pool(name="psg", bufs=1, space="PSUM") as psg,
        ):
            # ---- load constants -------------------------------------------
            lat = [singles.tile([128, BS], f16, tag=f"lat{j}", name=f"lat{j}")
                   for j in range(nkl)]
            for j in range(nkl):
                nc.sync.dma_start(out=lat[j], in_=latT_d[128 * j:128 * (j + 1), :])
            wd = [singles.tile([128, H], f16, tag=f"wd{j}", name=f"wd{j}")
                  for j in range(nkl)]
            for j in range(nkl):
                nc.sync.dma_start(out=wd[j], in_=wd_d[128 * j:128 * (j + 1), :])
            w = [singles.tile([128, H3], f16, tag=f"w{j}", name=f"w{j}")
                 for j in range(nkl)]
            for j in range(nkl):
                nc.sync.dma_start(out=w[j], in_=w_d[128 * j:128 * (j + 1), :])
            u = [singles.tile([128, H3], f16, tag=f"u{k}", name=f"u{k}")
                 for k in range(nk)]
            for k in range(nk):
                nc.sync.dma_start(out=u[k], in_=u_d[128 * k:128 * (k + 1), :])
            bx = singles.tile([1, H3], f16, tag="bx")
            nc.gpsimd.dma_start(out=bx, in_=bx_d[:, :])
            b1h = singles.tile([1, H], f16, tag="b1h")
            nc.gpsimd.dma_start(out=b1h, in_=b1h_d[:, :])
            bd = singles.tile([1, H], f16, tag="bd")
            nc.gpsimd.dma_start(out=bd, in_=bd_d[:, :])
            ones = singles.tile([1, 128], f16, tag="ones")
            nc.vector.memset(ones, 1.0)

            # persistent gate banks + 2 prologue scratch banks
            ps_z = psg.tile([128, H], f32, tag="ps_z")
            ps_r = psg.tile([128, H], f32, tag="ps_r")
            ps_h = psg.tile([128, H], f32, tag="ps_h")
            ps_a = psg.tile([128, H], f32, tag="ps_a")
            ps_b = psg.tile([128, H], f32, tag="ps_b")
            gate_ps = {GZ: ps_z, GR: ps_r, GH: ps_h}

            def cs(m):
                return slice(128 * m, 128 * (m + 1))

            # ---- prologue --------------------------------------------------
            # PSUM start_tensor_calc lazily zeroes the WHOLE 2KB zero region
            # (= the full bank row), so each bank gets exactly ONE start=True
            # (its first matmul); every other matmul accumulates.  The first
            # write to each not-yet-touched region then replaces (pending
            # zero), later writes accumulate -- which is what we want.
            # h0 = tanh((latent @ Wd)^T + bd), stacked
            for m in range(nm):
                for j in range(nkl):
                    nc.tensor.matmul(ps_a[:, cs(m)], wd[j][:, cs(m)], lat[j],
                                     start=(j == 0 and m == 0), stop=False,
                                     skip_group_check=True)
                nc.tensor.matmul(ps_a[:, cs(m)], bd[:, cs(m)], ones,
                                 start=False, stop=True, skip_group_check=True)
            h = hpool.tile([128, H], f16, tag="h")
            nc.scalar.activation(h, ps_a, AF.Tanh)

            # xh = (latent @ W_h)^T + bx_h, stacked, fp16 in SBUF
            for m in range(nm):
                for j in range(nkl):
                    nc.tensor.matmul(ps_b[:, cs(m)], w[j][:, GH * H + 128 * m:
                                                          GH * H + 128 * (m + 1)],
                                     lat[j], start=(j == 0 and m == 0),
                                     stop=False, skip_group_check=True)
                nc.tensor.matmul(ps_b[:, cs(m)],
                                 bx[:, GH * H + 128 * m:GH * H + 128 * (m + 1)],
                                 ones, start=False, stop=True,
                                 skip_group_check=True)
            xh = singles.tile([128, H], f16, tag="xh")
            nc.scalar.copy(xh, ps_b)

            # gate banks: x-projection + bias + h0 @ U_g
            for g in (GZ, GR, GH):
                ps = gate_ps[g]
                first = [True]

                def mm(dst, lhsT, rhs, stop=False):
                    nc.tensor.matmul(dst, lhsT, rhs, start=first[0], stop=stop,
                                     skip_group_check=True)
                    first[0] = False

                for m in range(nm):
                    if g == GH:
                        # h gate: recurrent bias only (xh is separate)
                        mm(ps[:, cs(m)], b1h[:, cs(m)], ones)
                    else:
                        for j in range(nkl):
                            mm(ps[:, cs(m)],
                               w[j][:, g * H + 128 * m:g * H + 128 * (m + 1)],
                               lat[j])
                        mm(ps[:, cs(m)],
                           bx[:, g * H + 128 * m:g * H + 128 * (m + 1)],
                           ones)
                    for k in range(nk):
                        mm(ps[:, cs(m)],
                           u[k][:, g * H + 128 * m:g * H + 128 * (m + 1)],
                           h[:, cs(k)], stop=(k == nk - 1))

            # ---- steady-state T loop --------------------------------------
            dl_prev = None
            for t in range(T):
                # matmul round t: ps_g += dl_{t} @ U_g, one group per dl
                # chunk, fired as soon as that chunk exists.  Gate order:
                # r first (spine head), then z (zb needed mid-tail), then h.
                if dl_prev is not None:
                    mmb = cfg.get("mm_blocks")
                    if mmb == "GM":
                        mmb = MM_GM
                    elif mmb == "SMART":
                        mmb = MM_SMART
                    elif mmb == "GMT":
                        mmb = MM_GM_TAPER
                    if mmb:
                        blocks = [(GMAP[gc], ms, ks)
                                  for gc, ms, ks in mmb]
                    else:
                        blocks = []
                        for (lo, hi) in spans:
                            ks = tuple(range(lo // 128, hi // 128))
                            for g in gate_seq:
                                blocks.append((g, tuple(range(nm)), ks))
                    for g, ms, ks in blocks:
                        ps = gate_ps[g]
                        for m in ms:
                            for k in ks:
                                _lab(nc.tensor.matmul(
                                    ps[:, cs(m)],
                                    u[k][:, g * H + 128 * m:
                                         g * H + 128 * (m + 1)],
                                    dl_prev[:, cs(k)],
                                    start=False, stop=(k == nk - 1),
                                    skip_group_check=True),
                                    f"mm.{'zrh'[g]}.m{m}.k{k}")

                # tail t
                r = work.tile([128, H], f16, tag="r")
                zb = work.tile([128, H], f16, tag="zb")
                t1 = work.tile([128, H], f16, tag="t1")
                t2 = work.tile([128, H], f16, tag="t2")
                hh = work.tile([128, H], f16, tag="hh")
                e = work.tile([128, H], f16, tag="e")
                dl = dpool.tile([128, H], f16, tag="dl", name=f"dl{t % 3}")
                hn = hpool.tile([128, H], f16, tag="h", name=f"h{t % 3}")

                for ci, (lo, hi) in enumerate(sr_spans):
                    sp = slice(lo, hi)
                    _lab(nc.scalar.activation(r[:, sp], ps_r[:, sp],
                                              AF.Sigmoid), f"sr.c{ci}")
                for ci, (lo, hi) in enumerate(zspans):
                    sp = slice(lo, hi)
                    _lab(nc.scalar.activation(zb[:, sp], ps_z[:, sp],
                                              AF.Sigmoid, scale=-1.0),
                         f"zb.c{ci}")
                for ci, (lo, hi) in enumerate(t_spans):
                    sp = slice(lo, hi)
                    _lab(nc.vector.tensor_tensor(out=t1[:, sp], in0=r[:, sp],
                                                 in1=ps_h[:, sp], op=OP.mult),
                         f"t1.c{ci}")
                    _lab(nc.vector.tensor_tensor(out=t2[:, sp], in0=t1[:, sp],
                                                 in1=xh[:, sp], op=OP.add),
                         f"t2.c{ci}")
                for ci, (lo, hi) in enumerate(th_spans):
                    sp = slice(lo, hi)
                    _lab(nc.scalar.activation(hh[:, sp], t2[:, sp], AF.Tanh),
                         f"th.c{ci}")
                for ci, (lo, hi) in enumerate(e_spans):
                    sp = slice(lo, hi)
                    _lab(nc.vector.tensor_tensor(out=e[:, sp], in0=hh[:, sp],
                                                 in1=h[:, sp],
                                                 op=OP.subtract), f"e.c{ci}")
                for ci, (lo, hi) in enumerate(dl_spans):
                    sp = slice(lo, hi)
                    _lab(nc.vector.tensor_tensor(out=dl[:, sp], in0=zb[:, sp],
                                                 in1=e[:, sp], op=OP.mult),
                         f"dl.c{ci}")
                for ci, (lo, hi) in enumerate(spans):
                    sp = slice(lo, hi)
                    _lab(nc.gpsimd.tensor_tensor(out=hn[:, sp], in0=h[:, sp],
                                                 in1=dl[:, sp], op=OP.add),
                         f"hn.c{ci}")

                _lab(nc.sync.dma_start(out=out_d[t], in_=hn), "dma.out")
                dl_prev = dl
                h = hn

    nc.compile()
    return nc



DUAL_CFG = dict(dma_alt=False, split_emit=False, sr_halves=False)


def _build_dual(T, cfg=None):
    dcfg = dict(DUAL_CFG)
    if cfg:
        dcfg.update(cfg)
    """Two independent 64-batch chains per core; each chain's spine is
    hidden behind the other's engine work.  Per-chain tiles are [128, 256]
    stacked as (m, b64): S[p, 64m+b] = X[b, 128m+p]."""
    import concourse.bass as bass
    import concourse.mybir as mybir
    import concourse.tile as tile
    from concourse import bacc

    f32 = mybir.dt.float32
    f16 = mybir.dt.float16
    AF = mybir.ActivationFunctionType
    OP = mybir.AluOpType

    nc = bacc.Bacc(None, target_bir_lowering=False, debug=False)

    latT_d = nc.dram_tensor("latT", [LD, BS], f16, kind="ExternalInput")
    wd_d = nc.dram_tensor("wd", [LD, H], f16, kind="ExternalInput")
    w_d = nc.dram_tensor("w", [LD, H3], f16, kind="ExternalInput")
    u_d = nc.dram_tensor("u", [H, H3], f16, kind="ExternalInput")
    bx_d = nc.dram_tensor("bx", [1, H3], f16, kind="ExternalInput")
    b1h_d = nc.dram_tensor("b1h", [1, H], f16, kind="ExternalInput")
    bd_d = nc.dram_tensor("bd", [1, H], f16, kind="ExternalInput")
    # out[t, p, 256*c + 64*m + b] = h_{t+1}[64c + b, 128*m + p]
    out_d = nc.dram_tensor("out", [T, 128, H], f16, kind="ExternalOutput")

    nk = H // 128
    nm = H // 128
    nkl = LD // 128
    GZ, GR, GH = 0, 1, 2
    W2 = 64  # batch per chain

    def gcols(g, m):
        return slice(g * H + 128 * m, g * H + 128 * (m + 1))

    with tile.TileContext(nc) as tc:
        with (
            tc.tile_pool(name="singles", bufs=1) as singles,
            tc.tile_pool(name="hpool", bufs=3) as hpool,
            tc.tile_pool(name="dpool", bufs=3) as dpool,
            tc.tile_pool(name="work", bufs=3) as work,
            tc.tile_pool(name="psg", bufs=1, space="PSUM") as psg,
        ):
            lat = [singles.tile([128, BS], f16, tag=f"lat{j}", name=f"lat{j}")
                   for j in range(nkl)]
            for j in range(nkl):
                nc.sync.dma_start(out=lat[j], in_=latT_d[128 * j:128 * (j + 1), :])
            wd = [singles.tile([128, H], f16, tag=f"wd{j}", name=f"wd{j}")
                  for j in range(nkl)]
            for j in range(nkl):
                nc.sync.dma_start(out=wd[j], in_=wd_d[128 * j:128 * (j + 1), :])
            w = [singles.tile([128, H3], f16, tag=f"w{j}", name=f"w{j}")
                 for j in range(nkl)]
            for j in range(nkl):
                nc.sync.dma_start(out=w[j], in_=w_d[128 * j:128 * (j + 1), :])
            u = [singles.tile([128, H3], f16, tag=f"u{k}", name=f"u{k}")
                 for k in range(nk)]
            for k in range(nk):
                nc.sync.dma_start(out=u[k], in_=u_d[128 * k:128 * (k + 1), :])
            bx = singles.tile([1, H3], f16, tag="bx")
            nc.gpsimd.dma_start(out=bx, in_=bx_d[:, :])
            b1h = singles.tile([1, H], f16, tag="b1h")
            nc.gpsimd.dma_start(out=b1h, in_=b1h_d[:, :])
            bd = singles.tile([1, H], f16, tag="bd")
            nc.gpsimd.dma_start(out=bd, in_=bd_d[:, :])
            ones = singles.tile([1, 128], f16, tag="ones")
            nc.vector.memset(ones, 1.0)

            # PSUM: full banks; chains use [:, 0:256].  8 banks total.
            names = ["pza", "pra", "pha", "pzb", "prb", "phb", "psa", "psb"]
            banks = {n: psg.tile([128, H], f32, tag=n, name=n) for n in names}
            gate_ps = {
                0: {GZ: banks["pza"], GR: banks["pra"], GH: banks["pha"]},
                1: {GZ: banks["pzb"], GR: banks["prb"], GH: banks["phb"]},
            }

            def bsl(m):
                # per-chain stacked free slice for h-chunk m
                return slice(W2 * m, W2 * (m + 1))

            chains = []
            ps_first = {n: [True] for n in names}

            def mm(bank_name, dst, lhsT, rhs, stop=False):
                f = ps_first[bank_name]
                nc.tensor.matmul(dst, lhsT, rhs, start=f[0], stop=stop,
                                 skip_group_check=True)
                f[0] = False

            # ---- prologue (both chains) -------------------------------
            hs = []
            xhs = []
            for c in range(2):
                bs = slice(W2 * c, W2 * (c + 1))  # batch cols in lat tiles
                po = 256 * c  # offset into shared prologue banks
                ps_a, ps_b = banks["psa"], banks["psb"]
                for m in range(nm):
                    for j in range(nkl):
                        mm("psa", ps_a[:, po + W2 * m: po + W2 * (m + 1)],
                           wd[j][:, 128 * m:128 * (m + 1)], lat[j][:, bs])
                    mm("psa", ps_a[:, po + W2 * m: po + W2 * (m + 1)],
                       bd[:, 128 * m:128 * (m + 1)], ones[:, 0:W2],
                       stop=(m == nm - 1))
                h = hpool.tile([128, 4 * W2], f16, tag=f"h{c}",
                               name=f"h_{c}_init")
                nc.scalar.activation(h, ps_a[:, po:po + 256], AF.Tanh)

                for m in range(nm):
                    for j in range(nkl):
                        mm("psb", ps_b[:, po + W2 * m: po + W2 * (m + 1)],
                           w[j][:, gcols(GH, m)], lat[j][:, bs])
                    mm("psb", ps_b[:, po + W2 * m: po + W2 * (m + 1)],
                       bx[:, gcols(GH, m)], ones[:, 0:W2], stop=(m == nm - 1))
                xh = singles.tile([128, 4 * W2], f16, tag=f"xh{c}")
                nc.scalar.copy(xh, ps_b[:, po:po + 256])
                hs.append(h)
                xhs.append(xh)

                for g in (GZ, GR, GH):
                    ps = gate_ps[c][g]
                    bn = names[c * 3 + [GZ, GR, GH].index(g)]
                    for m in range(nm):
                        if g == GH:
                            mm(bn, ps[:, bsl(m)], b1h[:, 128 * m:128 * (m + 1)],
                               ones[:, 0:W2])
                        else:
                            for j in range(nkl):
                                mm(bn, ps[:, bsl(m)], w[j][:, gcols(g, m)],
                                   lat[j][:, bs])
                            mm(bn, ps[:, bsl(m)], bx[:, gcols(g, m)],
                               ones[:, 0:W2])
                        for k in range(nk):
                            mm(bn, ps[:, bsl(m)], u[k][:, gcols(g, m)],
                               h[:, bsl(k)], stop=(k == nk - 1))

            # ---- T loop: software-pipelined chain interleave ----------
            # Emission order (per t):  s1(A,t)  s2(B,t-1)  s1(B,t)  s2(A,t)
            # so each chain's tanh is immediately followed (in per-engine
            # program order) by its own next-step sigmoid, matching the
            # half-cycle phase offset between the chains.
            dls = [None, None]
            pend = [None, None]

            def s1(c, t):
                # matmul round (consumes dls[c]) + sr/zb/t1/t2
                if dls[c] is not None:
                    for g in (GR, GH, GZ):
                        ps = gate_ps[c][g]
                        for m in range(nm):
                            for k in range(nk):
                                _lab(nc.tensor.matmul(
                                    ps[:, bsl(m)], u[k][:, gcols(g, m)],
                                    dls[c][:, bsl(k)], start=False,
                                    stop=(k == nk - 1),
                                    skip_group_check=True),
                                    f"mm{c}.{'zrh'[g]}.m{m}.k{k}")
                ps_z, ps_r, ps_h = (gate_ps[c][GZ], gate_ps[c][GR],
                                    gate_ps[c][GH])
                r = work.tile([128, 256], f16, tag=f"r{c}")
                zb = work.tile([128, 256], f16, tag=f"zb{c}")
                t1 = work.tile([128, 256], f16, tag=f"t1{c}")
                t2 = work.tile([128, 256], f16, tag=f"t2{c}")
                _lab(nc.scalar.activation(r, ps_r[:, 0:256], AF.Sigmoid),
                     f"sr{c}")
                _lab(nc.scalar.activation(zb, ps_z[:, 0:256], AF.Sigmoid,
                                          scale=-1.0), f"zb{c}")
                _lab(nc.vector.tensor_tensor(out=t1, in0=r,
                                             in1=ps_h[:, 0:256],
                                             op=OP.mult), f"t1{c}")
                _lab(nc.vector.tensor_tensor(out=t2, in0=t1, in1=xhs[c],
                                             op=OP.add), f"t2{c}")
                return (t, zb, t2)

            def s2(c):
                t, zb, t2 = pend[c]
                h = hs[c]
                hh = work.tile([128, 256], f16, tag=f"hh{c}")
                e = work.tile([128, 256], f16, tag=f"e{c}")
                dl = dpool.tile([128, 256], f16, tag=f"dl{c}",
                                name=f"dl{c}_{t % 3}")
                hn = hpool.tile([128, 256], f16, tag=f"h{c}",
                                name=f"h{c}_{t % 3}")
                _lab(nc.scalar.activation(hh, t2, AF.Tanh), f"th{c}")
                _lab(nc.vector.tensor_tensor(out=e, in0=hh, in1=h,
                                             op=OP.subtract), f"e{c}")
                _lab(nc.vector.tensor_tensor(out=dl, in0=zb, in1=e,
                                             op=OP.mult), f"dl{c}")
                _lab(nc.gpsimd.tensor_tensor(out=hn, in0=h, in1=dl,
                                             op=OP.add), f"hn{c}")
                dma_eng = nc.gpsimd if (dcfg["dma_alt"] and c == 1) \
                    else nc.sync
                _lab(dma_eng.dma_start(out=out_d[t][:, 256 * c:256 * (c + 1)],
                                       in_=hn), f"dma{c}")
                dls[c] = dl
                hs[c] = hn

            for t in range(T):
                pend[0] = s1(0, t)
                if pend[1] is not None:
                    s2(1)
                pend[1] = s1(1, t)
                s2(0)
            s2(1)

    nc.compile()
    return nc


def kernel(latent, Wd, bd, W, U, b, T, _trace=False):
    from concourse.bass_utils import run_bass_kernel_spmd

    latent = np.asarray(latent, dtype=np.float32)
    Wd = np.asarray(Wd, dtype=np.float32)
    bd = np.asarray(bd, dtype=np.float32)
    W = np.asarray(W, dtype=np.float32)
    U = np.asarray(U, dtype=np.float32)
    b = np.asarray(b, dtype=np.float32)
    T = int(T)

    key = (T,)
    if key not in _BUILD_CACHE:
        _BUILD_CACHE[key] = _build_dual(T) if ARCH == "dual" else _build(T)
    nc = _BUILD_CACHE[key]

    bx = b[0].copy()
    bx[: 2 * H] += b[1][: 2 * H]
    bx16 = np.ascontiguousarray(bx.reshape(1, H3)).astype(np.float16)
    b1h16 = np.ascontiguousarray(b[1][2 * H:].reshape(1, H)).astype(np.float16)
    bd16 = np.ascontiguousarray(bd.reshape(1, H)).astype(np.float16)
    u16 = np.ascontiguousarray(U).astype(np.float16)
    w16 = np.ascontiguousarray(W).astype(np.float16)
    wd16 = np.ascontiguousarray(Wd).astype(np.float16)

    in_maps = []
    for c in range(NCORES):
        rows = slice(c * BS, (c + 1) * BS)
        in_maps.append({
            "latT": np.ascontiguousarray(latent[rows].T).astype(np.float16),
            "wd": wd16, "w": w16, "u": u16,
            "bx": bx16, "b1h": b1h16, "bd": bd16,
        })

    res = run_bass_kernel_spmd(nc, in_maps, core_ids=list(range(NCORES)),
                               trace=_trace)
    if _trace and res.exec_time_ns is not None:
        print(f"HW exec time: {res.exec_time_ns} ns")
        if res.instructions_and_trace is not None:
            print(f"trace: {res.instructions_and_trace[1]}")

    # device output is stacked-transposed fp16; reassemble per arch.
    outs = []
    for c in range(NCORES):
        arr = res.results[c]["out"]  # [T, 128, 512] fp16
        if ARCH == "dual":
            # arr[t, p, 256*cc + 64*m + b] = h_{t+1}[64*cc + b, 128*m + p]
            parts = []
            for cc in range(2):
                sub = arr[:, :, 256 * cc:256 * (cc + 1)]
                sub = sub.reshape(T, 128, H // 128, 64).transpose(3, 0, 2, 1)
                parts.append(sub.reshape(64, T, H))
            outs.append(np.concatenate(parts, axis=0))
        else:
            # arr[t, p, 128*m + b] = h_{t+1}[b, 128*m + p]
            arr = arr.reshape(T, 128, H // 128, 128).transpose(3, 0, 2, 1)
            outs.append(arr.reshape(BS, T, H))
    return np.ascontiguousarray(np.concatenate(outs, axis=0)).astype(np.float32)


# revision 18
# speedup vs baseline: 1.7879x; 1.1629x over previous
# GRU decoder kernel for Trainium2 (Bass/Tile), data-parallel over batch.
#
# Problem (per reference):
#   h0 = tanh(latent @ Wd + bd)                      [B, H]
#   x  = latent @ W + b[0]; xz, xr, xh = split(x, 3) [B, 3H]
#   for t in range(T):   (reset_after GRU, recurrent bias b[1])
#       rec = h @ U + b[1]; rz, rr, rh = split(rec, 3)
#       z = sigmoid(xz + rz); r = sigmoid(xr + rr)
#       hh = tanh(xh + r * rh)
#       h = z*h + (1-z)*hh        -> out[:, t, :]
#
# Sharding: batch 1024 -> 8 cores x 128 rows, weights replicated, T loop
# local per core (no collectives).
#
# Layout: everything lives TRANSPOSED and "chunk-stacked": a [B, H] tensor
# is stored as a [128, 512] tile S with S[p, 128*m + b] = X[b, 128*m + p]
# (m = H-chunk, p = row within chunk, b = batch row). In this layout the
# hidden state IS the matmul moving operand (no transposes, no hT copies):
#   ps_g[:, 128m:128(m+1)] += U[128k:128(k+1), 512g+128m:...]^T @ h[:, 128k:...]
#
# Delta-accumulation: the three gate PSUM banks persistently hold
#   ps_z = xz + b1z + h_t@Uz,  ps_r = xr + b1r + h_t@Ur,  ps_h = b1h + h_t@Uh
# initialized once in the prologue; each step accumulates only
# dl = h_{t+1} - h_t through U (start=False), so there is no per-step
# bias/x-projection cost on any engine.  Matmul groups fire per dl chunk
# (k-chunks of that tail chunk) the moment the chunk is ready, ordered
# r-gate first (spine head), then z (zb is needed mid-tail), then h.
#
# Step tail per chunk (fp16):
#   r  = sigmoid(ps_r)            [ACT]
#   zb = sigmoid(-ps_z) (= 1-z)   [ACT]
#   t1 = r * ps_h                 [DVE]
#   t2 = t1 + xh                  [DVE]
#   hh = tanh(t2)                 [ACT]
#   e  = hh - h                   [DVE]
#   dl = zb * e   (= h' - h)      [DVE] -> feeds next matmul round
#   h' = h + dl                   [Pool] -> DMA out (fp16, host converts)

import numpy as np

B, LD, H, T_DEF = 1024, 256, 512, 128
H3 = 3 * H
NCORES = 8
BS = B // NCORES  # 128 batch rows per core

_BUILD_CACHE = {}
_LABELS = {}
ARCH = "dual"  # "dual" (N independent batch chains) or "single"
WIDTHS = (48, 40, 40)  # batch rows per chain (dual arch)


def _lab(r, s):
    try:
        _LABELS[r.ins.name] = s
    except Exception:
        pass
    return r

# tail chunk boundaries in the stacked free dim (multiples of 128)
CFG = dict(
    chunks=(0, 256, 512),    # matmul-group split (dl chunks that fire groups)
    sr_chunks=(0, 256, 512),  # sigmoid(r) op split
    zb_chunks=(0, 256, 512),  # zb op split
    t_chunks=(0, 256, 512),   # t1/t2 op split
    th_chunks=(0, 256, 512),  # tanh op split
    e_chunks=(0, 256, 512),   # e op split
    dl_chunks=(0, 256, 512),  # dl op split (>= mm chunk granularity)
    gate_order="rzh",        # gate order inside each matmul group
    mm_blocks="GM",          # explicit (gate, mset, kset) block list
)


# "smart" block order for the (0,256,512) split: r both halves first
# (spine head), then h/z interleaved by m-subset in due-time order.
MM_SMART = [
    ("r", (0, 1, 2, 3), (0, 1)),
    ("r", (0, 1), (2, 3)), ("r", (2, 3), (2, 3)),
    ("h", (0, 1, 2, 3), (0, 1)),
    ("h", (0, 1), (2, 3)),
    ("z", (0, 1), (0, 1)), ("z", (0, 1), (2, 3)),
    ("h", (2, 3), (2, 3)),
    ("z", (2, 3), (0, 1)), ("z", (2, 3), (2, 3)),
]

# gate-major: r both halves, h both, z both (in-order PE friendly)
MM_GM = [
    ("r", (0, 1, 2, 3), (0, 1)),
    ("r", (0, 1), (2, 3)), ("r", (2, 3), (2, 3)),
    ("h", (0, 1, 2, 3), (0, 1)),
    ("h", (0, 1), (2, 3)), ("h", (2, 3), (2, 3)),
    ("z", (0, 1, 2, 3), (0, 1)),
    ("z", (0, 1), (2, 3)), ("z", (2, 3), (2, 3)),
]

# taper (0,384,512): k012 groups fired by the big chunk, k3 by the small
MM_GM_TAPER = [
    ("r", (0, 1, 2, 3), (0, 1, 2)),
    ("r", (0, 1), (3,)), ("r", (2, 3), (3,)),
    ("h", (0, 1, 2, 3), (0, 1, 2)),
    ("h", (0, 1), (3,)), ("h", (2, 3), (3,)),
    ("z", (0, 1, 2, 3), (0, 1, 2)),
    ("z", (0, 1), (3,)), ("z", (2, 3), (3,)),
]


def _build(T, cfg=CFG):
    import concourse.bass as bass
    import concourse.mybir as mybir
    import concourse.tile as tile
    from concourse import bacc

    f32 = mybir.dt.float32
    f16 = mybir.dt.float16
    AF = mybir.ActivationFunctionType
    OP = mybir.AluOpType

    nc = bacc.Bacc(None, target_bir_lowering=False, debug=False)

    latT_d = nc.dram_tensor("latT", [LD, BS], f16, kind="ExternalInput")
    wd_d = nc.dram_tensor("wd", [LD, H], f16, kind="ExternalInput")
    w_d = nc.dram_tensor("w", [LD, H3], f16, kind="ExternalInput")
    u_d = nc.dram_tensor("u", [H, H3], f16, kind="ExternalInput")
    # bx = b[0] with b[1] z/r parts folded in; b1h = b[1] h third; bd
    bx_d = nc.dram_tensor("bx", [1, H3], f16, kind="ExternalInput")
    b1h_d = nc.dram_tensor("b1h", [1, H], f16, kind="ExternalInput")
    bd_d = nc.dram_tensor("bd", [1, H], f16, kind="ExternalInput")
    out_d = nc.dram_tensor("out", [T, 128, H], f16, kind="ExternalOutput")

    nk = H // 128    # 4 k-chunks over hidden
    nm = H // 128    # 4 m-chunks per gate
    nkl = LD // 128  # 2 k-chunks over latent
    # gate order in U/W columns (reference): z=0, r=1, h=2
    GZ, GR, GH = 0, 1, 2
    GMAP = {"r": GR, "z": GZ, "h": GH}
    gate_seq = [GMAP[ch] for ch in cfg["gate_order"]]
    spans = list(zip(cfg["chunks"][:-1], cfg["chunks"][1:]))

    def _spans(key):
        c = cfg.get(key, cfg["chunks"])
        return list(zip(c[:-1], c[1:]))

    sr_spans = _spans("sr_chunks")
    zspans = _spans("zb_chunks")
    t_spans = _spans("t_chunks")
    th_spans = _spans("th_chunks")
    e_spans = _spans("e_chunks")
    dl_spans = _spans("dl_chunks")

    with tile.TileContext(nc) as tc:
        with (
            tc.tile_pool(name="singles", bufs=1) as singles,
            tc.tile_pool(name="hpool", bufs=3) as hpool,
            tc.tile_pool(name="dpool", bufs=3) as dpool,
            tc.tile_pool(name="work", bufs=3) as work,
            tc.tile_pool(name="psg", bufs=1, space="PSUM") as psg,
        ):
            # ---- load constants -------------------------------------------
            lat = [singles.tile([128, BS], f16, tag=f"lat{j}", name=f"lat{j}")
                   for j in range(nkl)]
            for j in range(nkl):
                nc.sync.dma_start(out=lat[j], in_=latT_d[128 * j:128 * (j + 1), :])
            wd = [singles.tile([128, H], f16, tag=f"wd{j}", name=f"wd{j}")
                  for j in range(nkl)]
            for j in range(nkl):
                nc.sync.dma_start(out=wd[j], in_=wd_d[128 * j:128 * (j + 1), :])
            w = [singles.tile([128, H3], f16, tag=f"w{j}", name=f"w{j}")
                 for j in range(nkl)]
            for j in range(nkl):
                nc.sync.dma_start(out=w[j], in_=w_d[128 * j:128 * (j + 1), :])
            u = [singles.tile([128, H3], f16, tag=f"u{k}", name=f"u{k}")
                 for k in range(nk)]
            for k in range(nk):
                nc.sync.dma_start(out=u[k], in_=u_d[128 * k:128 * (k + 1), :])
            bx = singles.tile([1, H3], f16, tag="bx")
            nc.gpsimd.dma_start(out=bx, in_=bx_d[:, :])
            b1h = singles.tile([1, H], f16, tag="b1h")
            nc.gpsimd.dma_start(out=b1h, in_=b1h_d[:, :])
            bd = singles.tile([1, H], f16, tag="bd")
            nc.gpsimd.dma_start(out=bd, in_=bd_d[:, :])
            ones = singles.tile([1, 128], f16, tag="ones")
            nc.vector.memset(ones, 1.0)

            # persistent gate banks + 2 prologue scratch banks
            ps_z = psg.tile([128, H], f32, tag="ps_z")
            ps_r = psg.tile([128, H], f32, tag="ps_r")
            ps_h = psg.tile([128, H], f32, tag="ps_h")
            ps_a = psg.tile([128, H], f32, tag="ps_a")
            ps_b = psg.tile([128, H], f32, tag="ps_b")
            gate_ps = {GZ: ps_z, GR: ps_r, GH: ps_h}

            def cs(m):
                return slice(128 * m, 128 * (m + 1))

            # ---- prologue --------------------------------------------------
            # PSUM start_tensor_calc lazily zeroes the WHOLE 2KB zero region
            # (= the full bank row), so each bank gets exactly ONE start=True
            # (its first matmul); every other matmul accumulates.  The first
            # write to each not-yet-touched region then replaces (pending
            # zero), later writes accumulate -- which is what we want.
            # h0 = tanh((latent @ Wd)^T + bd), stacked
            for m in range(nm):
                for j in range(nkl):
                    nc.tensor.matmul(ps_a[:, cs(m)], wd[j][:, cs(m)], lat[j],
                                     start=(j == 0 and m == 0), stop=False,
                                     skip_group_check=True)
                nc.tensor.matmul(ps_a[:, cs(m)], bd[:, cs(m)], ones,
                                 start=False, stop=True, skip_group_check=True)
            h = hpool.tile([128, H], f16, tag="h")
            nc.scalar.activation(h, ps_a, AF.Tanh)

            # xh = (latent @ W_h)^T + bx_h, stacked, fp16 in SBUF
            for m in range(nm):
                for j in range(nkl):
                    nc.tensor.matmul(ps_b[:, cs(m)], w[j][:, GH * H + 128 * m:
                                                          GH * H + 128 * (m + 1)],
                                     lat[j], start=(j == 0 and m == 0),
                                     stop=False, skip_group_check=True)
                nc.tensor.matmul(ps_b[:, cs(m)],
                                 bx[:, GH * H + 128 * m:GH * H + 128 * (m + 1)],
                                 ones, start=False, stop=True,
                                 skip_group_check=True)
            xh = singles.tile([128, H], f16, tag="xh")
            nc.scalar.copy(xh, ps_b)

            # gate banks: x-projection + bias + h0 @ U_g
            for g in (GZ, GR, GH):
                ps = gate_ps[g]
                first = [True]

                def mm(dst, lhsT, rhs, stop=False):
                    nc.tensor.matmul(dst, lhsT, rhs, start=first[0], stop=stop,
                                     skip_group_check=True)
                    first[0] = False

                for m in range(nm):
                    if g == GH:
                        # h gate: recurrent bias only (xh is separate)
                        mm(ps[:, cs(m)], b1h[:, cs(m)], ones)
                    else:
                        for j in range(nkl):
                            mm(ps[:, cs(m)],
                               w[j][:, g * H + 128 * m:g * H + 128 * (m + 1)],
                               lat[j])
                        mm(ps[:, cs(m)],
                           bx[:, g * H + 128 * m:g * H + 128 * (m + 1)],
                           ones)
                    for k in range(nk):
                        mm(ps[:, cs(m)],
                           u[k][:, g * H + 128 * m:g * H + 128 * (m + 1)],
                           h[:, cs(k)], stop=(k == nk - 1))

            # ---- steady-state T loop --------------------------------------
            dl_prev = None
            for t in range(T):
                # matmul round t: ps_g += dl_{t} @ U_g, one group per dl
                # chunk, fired as soon as that chunk exists.  Gate order:
                # r first (spine head), then z (zb needed mid-tail), then h.
                if dl_prev is not None:
                    mmb = cfg.get("mm_blocks")
                    if mmb == "GM":
                        mmb = MM_GM
                    elif mmb == "SMART":
                        mmb = MM_SMART
                    elif mmb == "GMT":
                        mmb = MM_GM_TAPER
                    if mmb:
                        blocks = [(GMAP[gc], ms, ks)
                                  for gc, ms, ks in mmb]
                    else:
                        blocks = []
                        for (lo, hi) in spans:
                            ks = tuple(range(lo // 128, hi // 128))
                            for g in gate_seq:
                                blocks.append((g, tuple(range(nm)), ks))
                    for g, ms, ks in blocks:
                        ps = gate_ps[g]
                        for m in ms:
                            for k in ks:
                                _lab(nc.tensor.matmul(
                                    ps[:, cs(m)],
                                    u[k][:, g * H + 128 * m:
                                         g * H + 128 * (m + 1)],
                                    dl_prev[:, cs(k)],
                                    start=False, stop=(k == nk - 1),
                                    skip_group_check=True),
                                    f"mm.{'zrh'[g]}.m{m}.k{k}")

                # tail t
                r = work.tile([128, H], f16, tag="r")
                zb = work.tile([128, H], f16, tag="zb")
                t1 = work.tile([128, H], f16, tag="t1")
                t2 = work.tile([128, H], f16, tag="t2")
                hh = work.tile([128, H], f16, tag="hh")
                e = work.tile([128, H], f16, tag="e")
                dl = dpool.tile([128, H], f16, tag="dl", name=f"dl{t % 3}")
                hn = hpool.tile([128, H], f16, tag="h", name=f"h{t % 3}")

                for ci, (lo, hi) in enumerate(sr_spans):
                    sp = slice(lo, hi)
                    _lab(nc.scalar.activation(r[:, sp], ps_r[:, sp],
                                              AF.Sigmoid), f"sr.c{ci}")
                for ci, (lo, hi) in enumerate(zspans):
                    sp = slice(lo, hi)
                    _lab(nc.scalar.activation(zb[:, sp], ps_z[:, sp],
                                              AF.Sigmoid, scale=-1.0),
                         f"zb.c{ci}")
                for ci, (lo, hi) in enumerate(t_spans):
                    sp = slice(lo, hi)
                    _lab(nc.vector.tensor_tensor(out=t1[:, sp], in0=r[:, sp],
                                                 in1=ps_h[:, sp], op=OP.mult),
                         f"t1.c{ci}")
                    _lab(nc.vector.tensor_tensor(out=t2[:, sp], in0=t1[:, sp],
                                                 in1=xh[:, sp], op=OP.add),
                         f"t2.c{ci}")
                for ci, (lo, hi) in enumerate(th_spans):
                    sp = slice(lo, hi)
                    _lab(nc.scalar.activation(hh[:, sp], t2[:, sp], AF.Tanh),
                         f"th.c{ci}")
                for ci, (lo, hi) in enumerate(e_spans):
                    sp = slice(lo, hi)
                    _lab(nc.vector.tensor_tensor(out=e[:, sp], in0=hh[:, sp],
                                                 in1=h[:, sp],
                                                 op=OP.subtract), f"e.c{ci}")
                for ci, (lo, hi) in enumerate(dl_spans):
                    sp = slice(lo, hi)
                    _lab(nc.vector.tensor_tensor(out=dl[:, sp], in0=zb[:, sp],
                                                 in1=e[:, sp], op=OP.mult),
                         f"dl.c{ci}")
                for ci, (lo, hi) in enumerate(spans):
                    sp = slice(lo, hi)
                    _lab(nc.gpsimd.tensor_tensor(out=hn[:, sp], in0=h[:, sp],
                                                 in1=dl[:, sp], op=OP.add),
                         f"hn.c{ci}")

                _lab(nc.sync.dma_start(out=out_d[t], in_=hn), "dma.out")
                dl_prev = dl
                h = hn

    nc.compile()
    return nc



DUAL_CFG = dict(dma_alt=False, split_emit=False, sr_halves=False,
                widths=WIDTHS, rz_merge=False)


def _build_dual(T, cfg=None):
    dcfg = dict(DUAL_CFG)
    if cfg:
        dcfg.update(cfg)
    widths = list(dcfg["widths"])
    NCH = len(widths)
    offs = [sum(widths[:i]) for i in range(NCH)]          # batch col offsets
    soffs = [4 * o for o in offs]                          # stacked col offsets
    """Two independent 64-batch chains per core; each chain's spine is
    hidden behind the other's engine work.  Per-chain tiles are [128, 256]
    stacked as (m, b64): S[p, 64m+b] = X[b, 128m+p]."""
    import concourse.bass as bass
    import concourse.mybir as mybir
    import concourse.tile as tile
    from concourse import bacc

    f32 = mybir.dt.float32
    f16 = mybir.dt.float16
    AF = mybir.ActivationFunctionType
    OP = mybir.AluOpType

    nc = bacc.Bacc(None, target_bir_lowering=False, debug=False)

    latT_d = nc.dram_tensor("latT", [LD, BS], f16, kind="ExternalInput")
    wd_d = nc.dram_tensor("wd", [LD, H], f16, kind="ExternalInput")
    w_d = nc.dram_tensor("w", [LD, H3], f16, kind="ExternalInput")
    u_d = nc.dram_tensor("u", [H, H3], f16, kind="ExternalInput")
    bx_d = nc.dram_tensor("bx", [1, H3], f16, kind="ExternalInput")
    b1h_d = nc.dram_tensor("b1h", [1, H], f16, kind="ExternalInput")
    bd_d = nc.dram_tensor("bd", [1, H], f16, kind="ExternalInput")
    # out[t, p, 256*c + 64*m + b] = h_{t+1}[64c + b, 128*m + p]
    out_d = nc.dram_tensor("out", [T, 128, H], f16, kind="ExternalOutput")

    nk = H // 128
    nm = H // 128
    nkl = LD // 128
    GZ, GR, GH = 0, 1, 2

    def gcols(g, m):
        return slice(g * H + 128 * m, g * H + 128 * (m + 1))

    with tile.TileContext(nc) as tc:
        with (
            tc.tile_pool(name="singles", bufs=1) as singles,
            tc.tile_pool(name="hpool", bufs=3) as hpool,
            tc.tile_pool(name="dpool", bufs=3) as dpool,
            tc.tile_pool(name="work", bufs=3) as work,
            tc.tile_pool(name="psg", bufs=1, space="PSUM") as psg,
        ):
            lat = [singles.tile([128, BS], f16, tag=f"lat{j}", name=f"lat{j}")
                   for j in range(nkl)]
            for j in range(nkl):
                nc.sync.dma_start(out=lat[j], in_=latT_d[128 * j:128 * (j + 1), :])
            wd = [singles.tile([128, H], f16, tag=f"wd{j}", name=f"wd{j}")
                  for j in range(nkl)]
            for j in range(nkl):
                nc.sync.dma_start(out=wd[j], in_=wd_d[128 * j:128 * (j + 1), :])
            w = [singles.tile([128, H3], f16, tag=f"w{j}", name=f"w{j}")
                 for j in range(nkl)]
            for j in range(nkl):
                nc.sync.dma_start(out=w[j], in_=w_d[128 * j:128 * (j + 1), :])
            u = [singles.tile([128, H3], f16, tag=f"u{k}", name=f"u{k}")
                 for k in range(nk)]
            for k in range(nk):
                nc.sync.dma_start(out=u[k], in_=u_d[128 * k:128 * (k + 1), :])
            bx = singles.tile([1, H3], f16, tag="bx")
            nc.gpsimd.dma_start(out=bx, in_=bx_d[:, :])
            b1h = singles.tile([1, H], f16, tag="b1h")
            nc.gpsimd.dma_start(out=b1h, in_=b1h_d[:, :])
            bd = singles.tile([1, H], f16, tag="bd")
            nc.gpsimd.dma_start(out=bd, in_=bd_d[:, :])
            ones = singles.tile([1, 128], f16, tag="ones")
            nc.vector.memset(ones, 1.0)

            rz_merge = dcfg.get("rz_merge", False)
            if rz_merge:
                # per-chain bank holds [r-gate || z-gate] contiguously so ONE
                # sigmoid pass covers both; h-gate shared bank for all chains.
                names = [f"prz{c}" for c in range(NCH)] + ["ph", "psa", "psb"]
                banks = {n: psg.tile([128, H], f32, tag=n, name=n)
                         for n in names}
                gate_ps = {c: {GZ: banks[f"prz{c}"], GR: banks[f"prz{c}"],
                               GH: banks["ph"]} for c in range(NCH)}
                gate_bn = {}

                def bnof(c, g):
                    return "ph" if g == GH else f"prz{c}"

                def bsl(c, m, g=None):
                    W2 = widths[c]
                    if g == GH or g is None:
                        base = soffs[c]
                    elif g == GR:
                        base = 0
                    else:  # GZ
                        base = 4 * W2
                    return slice(base + W2 * m, base + W2 * (m + 1))
            else:
                # one bank per gate, shared by all chains via disjoint regions
                names = ["pz", "pr", "ph", "psa", "psb"]
                banks = {n: psg.tile([128, H], f32, tag=n, name=n)
                         for n in names}
                gate_ps = {c: {GZ: banks["pz"], GR: banks["pr"],
                               GH: banks["ph"]} for c in range(NCH)}

                def bnof(c, g):
                    return {GZ: "pz", GR: "pr", GH: "ph"}[g]

                def bsl(c, m, g=None):
                    W2 = widths[c]
                    return slice(soffs[c] + W2 * m, soffs[c] + W2 * (m + 1))

            ps_first = {n: [True] for n in names}

            def mm(bank_name, dst, lhsT, rhs, stop=False):
                f = ps_first[bank_name]
                nc.tensor.matmul(dst, lhsT, rhs, start=f[0], stop=stop,
                                 skip_group_check=True)
                f[0] = False

            # ---- prologue (all chains) --------------------------------
            hs = []
            xhs = []
            for c in range(NCH):
                W2 = widths[c]
                bs = slice(offs[c], offs[c] + W2)  # batch cols in lat tiles
                po = soffs[c]  # offset into shared prologue banks
                ps_a, ps_b = banks["psa"], banks["psb"]
                for m in range(nm):
                    for j in range(nkl):
                        mm("psa", ps_a[:, po + W2 * m: po + W2 * (m + 1)],
                           wd[j][:, 128 * m:128 * (m + 1)], lat[j][:, bs])
                    mm("psa", ps_a[:, po + W2 * m: po + W2 * (m + 1)],
                       bd[:, 128 * m:128 * (m + 1)], ones[:, 0:W2],
                       stop=(m == nm - 1))
                h = hpool.tile([128, 4 * W2], f16, tag=f"h{c}",
                               name=f"h_{c}_init")
                nc.scalar.activation(h, ps_a[:, po:po + 4 * W2], AF.Tanh)

                for m in range(nm):
                    for j in range(nkl):
                        mm("psb", ps_b[:, po + W2 * m: po + W2 * (m + 1)],
                           w[j][:, gcols(GH, m)], lat[j][:, bs])
                    mm("psb", ps_b[:, po + W2 * m: po + W2 * (m + 1)],
                       bx[:, gcols(GH, m)], ones[:, 0:W2], stop=(m == nm - 1))
                xh = singles.tile([128, 4 * W2], f16, tag=f"xh{c}")
                nc.scalar.copy(xh, ps_b[:, po:po + 4 * W2])
                hs.append(h)
                xhs.append(xh)

                for g in (GZ, GR, GH):
                    ps = gate_ps[c][g]
                    bn = bnof(c, g)
                    for m in range(nm):
                        if g == GH:
                            mm(bn, ps[:, bsl(c, m, g)],
                               b1h[:, 128 * m:128 * (m + 1)], ones[:, 0:W2])
                        else:
                            for j in range(nkl):
                                mm(bn, ps[:, bsl(c, m, g)],
                                   w[j][:, gcols(g, m)], lat[j][:, bs])
                            mm(bn, ps[:, bsl(c, m, g)], bx[:, gcols(g, m)],
                               ones[:, 0:W2])
                        for k in range(nk):
                            mm(bn, ps[:, bsl(c, m, g)], u[k][:, gcols(g, m)],
                               h[:, slice(W2 * k, W2 * (k + 1))],
                               stop=(k == nk - 1))

            # ---- T loop: software-pipelined chain interleave ----------
            # Emission order (per t):  s1(A,t)  s2(B,t-1)  s1(B,t)  s2(A,t)
            # so each chain's tanh is immediately followed (in per-engine
            # program order) by its own next-step sigmoid, matching the
            # half-cycle phase offset between the chains.
            dls = [None] * NCH
            pend = [None] * NCH

            def s1(c, t):
                W2 = widths[c]
                # matmul round (consumes dls[c]) + sigmoids + t1/t2
                gorder = (GR, GZ, GH) if rz_merge else (GR, GH, GZ)
                if dls[c] is not None:
                    for g in gorder:
                        ps = gate_ps[c][g]
                        for m in range(nm):
                            for k in range(nk):
                                _lab(nc.tensor.matmul(
                                    ps[:, bsl(c, m, g)], u[k][:, gcols(g, m)],
                                    dls[c][:, W2 * k:W2 * (k + 1)],
                                    start=False, stop=(k == nk - 1),
                                    skip_group_check=True),
                                    f"mm{c}.{'zrh'[g]}.m{m}.k{k}")
                ps_h = gate_ps[c][GH]
                hsl = slice(soffs[c], soffs[c] + 4 * W2)
                t1 = work.tile([128, 4 * W2], f16, tag=f"t1{c}")
                t2 = work.tile([128, 4 * W2], f16, tag=f"t2{c}")
                if rz_merge:
                    prz = banks[f"prz{c}"]
                    rz = work.tile([128, 8 * W2], f16, tag=f"rz{c}")
                    _lab(nc.scalar.activation(rz, prz[:, 0:8 * W2],
                                              AF.Sigmoid), f"srz{c}")
                    r = rz[:, 0:4 * W2]
                    zb = work.tile([128, 4 * W2], f16, tag=f"zb{c}")
                    _lab(nc.vector.tensor_scalar(
                        out=zb, in0=rz[:, 4 * W2:8 * W2], scalar1=-1.0,
                        scalar2=1.0, op0=OP.mult, op1=OP.add), f"zbv{c}")
                else:
                    ps_z, ps_r = gate_ps[c][GZ], gate_ps[c][GR]
                    r = work.tile([128, 4 * W2], f16, tag=f"r{c}")
                    zb = work.tile([128, 4 * W2], f16, tag=f"zb{c}")
                    _lab(nc.scalar.activation(r, ps_r[:, bsl(c, 0, GR).start:
                                                    bsl(c, 3, GR).stop],
                                              AF.Sigmoid), f"sr{c}")
                    _lab(nc.scalar.activation(zb, ps_z[:, bsl(c, 0, GZ).start:
                                                     bsl(c, 3, GZ).stop],
                                              AF.Sigmoid, scale=-1.0),
                         f"zb{c}")
                _lab(nc.vector.tensor_tensor(out=t1, in0=r,
                                             in1=ps_h[:, hsl],
                                             op=OP.mult), f"t1{c}")
                _lab(nc.vector.tensor_tensor(out=t2, in0=t1, in1=xhs[c],
                                             op=OP.add), f"t2{c}")
                return (t, zb, t2)

            def s2(c):
                W2 = widths[c]
                t, zb, t2 = pend[c]
                h = hs[c]
                hh = work.tile([128, 4 * W2], f16, tag=f"hh{c}")
                e = work.tile([128, 4 * W2], f16, tag=f"e{c}")
                dl = dpool.tile([128, 4 * W2], f16, tag=f"dl{c}",
                                name=f"dl{c}_{t % 3}")
                hn = hpool.tile([128, 4 * W2], f16, tag=f"h{c}",
                                name=f"h{c}_{t % 3}")
                _lab(nc.scalar.activation(hh, t2, AF.Tanh), f"th{c}")
                _lab(nc.vector.tensor_tensor(out=e, in0=hh, in1=h,
                                             op=OP.subtract), f"e{c}")
                _lab(nc.vector.tensor_tensor(out=dl, in0=zb, in1=e,
                                             op=OP.mult), f"dl{c}")
                _lab(nc.gpsimd.tensor_tensor(out=hn, in0=h, in1=dl,
                                             op=OP.add), f"hn{c}")
                dma_eng = nc.gpsimd if (dcfg["dma_alt"] and c % 2 == 1) \
                    else nc.sync
                _lab(dma_eng.dma_start(
                    out=out_d[t][:, soffs[c]:soffs[c] + 4 * W2],
                    in_=hn), f"dma{c}")
                dls[c] = dl
                hs[c] = hn

            for t in range(T):
                for c in range(NCH):
                    pend[c] = s1(c, t)
                    cprev = (c - 1) % NCH
                    if pend[cprev] is not None and (t > 0 or c > 0):
                        s2(cprev)
                        pend[cprev] = None
            for c in range(NCH):
                if pend[c] is not None:
                    s2(c)

    nc.compile()
    return nc


def kernel(latent, Wd, bd, W, U, b, T, _trace=False):
    from concourse.bass_utils import run_bass_kernel_spmd

    latent = np.asarray(latent, dtype=np.float32)
    Wd = np.asarray(Wd, dtype=np.float32)
    bd = np.asarray(bd, dtype=np.float32)
    W = np.asarray(W, dtype=np.float32)
    U = np.asarray(U, dtype=np.float32)
    b = np.asarray(b, dtype=np.float32)
    T = int(T)

    key = (T,)
    if key not in _BUILD_CACHE:
        _BUILD_CACHE[key] = _build_dual(T) if ARCH == "dual" else _build(T)
    nc = _BUILD_CACHE[key]

    bx = b[0].copy()
    bx[: 2 * H] += b[1][: 2 * H]
    bx16 = np.ascontiguousarray(bx.reshape(1, H3)).astype(np.float16)
    b1h16 = np.ascontiguousarray(b[1][2 * H:].reshape(1, H)).astype(np.float16)
    bd16 = np.ascontiguousarray(bd.reshape(1, H)).astype(np.float16)
    u16 = np.ascontiguousarray(U).astype(np.float16)
    w16 = np.ascontiguousarray(W).astype(np.float16)
    wd16 = np.ascontiguousarray(Wd).astype(np.float16)

    in_maps = []
    for c in range(NCORES):
        rows = slice(c * BS, (c + 1) * BS)
        in_maps.append({
            "latT": np.ascontiguousarray(latent[rows].T).astype(np.float16),
            "wd": wd16, "w": w16, "u": u16,
            "bx": bx16, "b1h": b1h16, "bd": bd16,
        })

    res = run_bass_kernel_spmd(nc, in_maps, core_ids=list(range(NCORES)),
                               trace=_trace)
    if _trace and res.exec_time_ns is not None:
        print(f"HW exec time: {res.exec_time_ns} ns")
        if res.instructions_and_trace is not None:
            print(f"trace: {res.instructions_and_trace[1]}")

    # device output is stacked-transposed fp16; reassemble per arch.
    outs = []
    for c in range(NCORES):
        arr = res.results[c]["out"]  # [T, 128, 512] fp16
        if ARCH == "dual":
            # chain cc: arr[t, p, 4*off + w*m + b] = h_{t+1}[off + b, 128m + p]
            parts = []
            off = 0
            for wch in WIDTHS:
                sub = arr[:, :, 4 * off:4 * (off + wch)]
                sub = sub.reshape(T, 128, H // 128, wch).transpose(3, 0, 2, 1)
                parts.append(sub.reshape(wch, T, H))
                off += wch
            outs.append(np.concatenate(parts, axis=0))
        else:
            # arr[t, p, 128*m + b] = h_{t+1}[b, 128*m + p]
            arr = arr.reshape(T, 128, H // 128, 128).transpose(3, 0, 2, 1)
            outs.append(arr.reshape(BS, T, H))
    return np.ascontiguousarray(np.concatenate(outs, axis=0)).astype(np.float32)


# revision 20
# speedup vs baseline: 1.7962x; 1.0047x over previous
# GRU decoder kernel for Trainium2 (Bass/Tile), data-parallel over batch.
#
# Problem (per reference):
#   h0 = tanh(latent @ Wd + bd)                      [B, H]
#   x  = latent @ W + b[0]; xz, xr, xh = split(x, 3) [B, 3H]
#   for t in range(T):   (reset_after GRU, recurrent bias b[1])
#       rec = h @ U + b[1]; rz, rr, rh = split(rec, 3)
#       z = sigmoid(xz + rz); r = sigmoid(xr + rr)
#       hh = tanh(xh + r * rh)
#       h = z*h + (1-z)*hh        -> out[:, t, :]
#
# Sharding: batch 1024 -> 8 cores x 128 rows, weights replicated, T loop
# local per core (no collectives).
#
# The kernel is LATENCY-bound (serial recurrence, 128 sequential steps),
# so the design minimizes the per-step dependency spine:
#
# 1. TRANSPOSED "chunk-stacked" layout: a [B, H] tensor is a [128, 4W]
#    tile S with S[p, W*m + b] = X[b, 128*m + p] (m = H-chunk, p = row in
#    chunk, b = batch row).  The hidden state IS the matmul moving operand
#    (contraction over H sits on partitions) -- no transposes, no copies:
#      ps_g[:, W*m:...] += U[128k:128(k+1), 512g+128m:...]^T @ h[:, W*k:...]
#
# 2. DELTA-accumulation: per gate, a persistent PSUM region holds
#    x_g + b_g + h_t@U_g, initialized once in the prologue; each step
#    accumulates only dl = h_{t+1} - h_t through U (start=False always;
#    PSUM start_tensor_calc lazily zeroes a whole 2KB zero region, so each
#    bank gets exactly ONE start=True on its first prologue matmul).  No
#    per-step bias or x-projection cost on any engine.
#
# 3. MULTI-CHAIN latency hiding: the core's 128 batch rows split into
#    NCH=3 independent recurrences (WIDTHS), phase-shifted ~C/3 apart; each
#    chain's serial spine (sigmoid -> r*ps_h -> +xh -> tanh -> (hh-h)*zb)
#    executes while the other chains occupy PE/ACT/DVE, keeping ACT ~85%
#    busy.  Emission is software-pipelined (s1 = mms+sigmoids+t1/t2,
#    s2 = tanh+e+dl+hn+dma) with a one-third-iteration skew.
#
# Step tail per chain (fp16, one op each, full chain width):
#   r  = sigmoid(ps_r)            [ACT]
#   zb = sigmoid(-ps_z) (= 1-z)   [ACT]
#   t1 = r * ps_h                 [DVE]
#   t2 = t1 + xh                  [DVE]
#   hh = tanh(t2)                 [ACT]
#   e  = hh - h                   [DVE]
#   dl = zb * e   (= h' - h)      [DVE] -> feeds next matmul round
#   h' = h + dl                   [Pool] -> DMA out (fp16, host converts)
#
# fp16 everywhere off-PSUM: weights/moving operands fp16 (1 cyc/row on PE),
# tail intermediates fp16 (DVE 2x / tensor_scalar 4x modes); PSUM stays
# f32.  Accumulated fp16 error ~2.5e-3 rel (threshold 2e-2).
# History: baseline 811 us -> single-chain delta/transposed 583 us ->
# 2 chains 528 us -> 3 chains ~452 us (TimelineSim).

import numpy as np

B, LD, H, T_DEF = 1024, 256, 512, 128
H3 = 3 * H
NCORES = 8
BS = B // NCORES  # 128 batch rows per core

_BUILD_CACHE = {}
_LABELS = {}
ARCH = "dual"  # "dual" (N independent batch chains) or "single"
WIDTHS = (40, 44, 44)  # batch rows per chain (dual arch)


def _lab(r, s):
    try:
        _LABELS[r.ins.name] = s
    except Exception:
        pass
    return r

# tail chunk boundaries in the stacked free dim (multiples of 128)
CFG = dict(
    chunks=(0, 256, 512),    # matmul-group split (dl chunks that fire groups)
    sr_chunks=(0, 256, 512),  # sigmoid(r) op split
    zb_chunks=(0, 256, 512),  # zb op split
    t_chunks=(0, 256, 512),   # t1/t2 op split
    th_chunks=(0, 256, 512),  # tanh op split
    e_chunks=(0, 256, 512),   # e op split
    dl_chunks=(0, 256, 512),  # dl op split (>= mm chunk granularity)
    gate_order="rzh",        # gate order inside each matmul group
    mm_blocks="GM",          # explicit (gate, mset, kset) block list
)


# "smart" block order for the (0,256,512) split: r both halves first
# (spine head), then h/z interleaved by m-subset in due-time order.
MM_SMART = [
    ("r", (0, 1, 2, 3), (0, 1)),
    ("r", (0, 1), (2, 3)), ("r", (2, 3), (2, 3)),
    ("h", (0, 1, 2, 3), (0, 1)),
    ("h", (0, 1), (2, 3)),
    ("z", (0, 1), (0, 1)), ("z", (0, 1), (2, 3)),
    ("h", (2, 3), (2, 3)),
    ("z", (2, 3), (0, 1)), ("z", (2, 3), (2, 3)),
]

# gate-major: r both halves, h both, z both (in-order PE friendly)
MM_GM = [
    ("r", (0, 1, 2, 3), (0, 1)),
    ("r", (0, 1), (2, 3)), ("r", (2, 3), (2, 3)),
    ("h", (0, 1, 2, 3), (0, 1)),
    ("h", (0, 1), (2, 3)), ("h", (2, 3), (2, 3)),
    ("z", (0, 1, 2, 3), (0, 1)),
    ("z", (0, 1), (2, 3)), ("z", (2, 3), (2, 3)),
]

# taper (0,384,512): k012 groups fired by the big chunk, k3 by the small
MM_GM_TAPER = [
    ("r", (0, 1, 2, 3), (0, 1, 2)),
    ("r", (0, 1), (3,)), ("r", (2, 3), (3,)),
    ("h", (0, 1, 2, 3), (0, 1, 2)),
    ("h", (0, 1), (3,)), ("h", (2, 3), (3,)),
    ("z", (0, 1, 2, 3), (0, 1, 2)),
    ("z", (0, 1), (3,)), ("z", (2, 3), (3,)),
]


def _build(T, cfg=CFG):
    import concourse.bass as bass
    import concourse.mybir as mybir
    import concourse.tile as tile
    from concourse import bacc

    f32 = mybir.dt.float32
    f16 = mybir.dt.float16
    AF = mybir.ActivationFunctionType
    OP = mybir.AluOpType

    nc = bacc.Bacc(None, target_bir_lowering=False, debug=False)

    latT_d = nc.dram_tensor("latT", [LD, BS], f16, kind="ExternalInput")
    wd_d = nc.dram_tensor("wd", [LD, H], f16, kind="ExternalInput")
    w_d = nc.dram_tensor("w", [LD, H3], f16, kind="ExternalInput")
    u_d = nc.dram_tensor("u", [H, H3], f16, kind="ExternalInput")
    # bx = b[0] with b[1] z/r parts folded in; b1h = b[1] h third; bd
    bx_d = nc.dram_tensor("bx", [1, H3], f16, kind="ExternalInput")
    b1h_d = nc.dram_tensor("b1h", [1, H], f16, kind="ExternalInput")
    bd_d = nc.dram_tensor("bd", [1, H], f16, kind="ExternalInput")
    out_d = nc.dram_tensor("out", [T, 128, H], f16, kind="ExternalOutput")

    nk = H // 128    # 4 k-chunks over hidden
    nm = H // 128    # 4 m-chunks per gate
    nkl = LD // 128  # 2 k-chunks over latent
    # gate order in U/W columns (reference): z=0, r=1, h=2
    GZ, GR, GH = 0, 1, 2
    GMAP = {"r": GR, "z": GZ, "h": GH}
    gate_seq = [GMAP[ch] for ch in cfg["gate_order"]]
    spans = list(zip(cfg["chunks"][:-1], cfg["chunks"][1:]))

    def _spans(key):
        c = cfg.get(key, cfg["chunks"])
        return list(zip(c[:-1], c[1:]))

    sr_spans = _spans("sr_chunks")
    zspans = _spans("zb_chunks")
    t_spans = _spans("t_chunks")
    th_spans = _spans("th_chunks")
    e_spans = _spans("e_chunks")
    dl_spans = _spans("dl_chunks")

    with tile.TileContext(nc) as tc:
        with (
            tc.tile_pool(name="singles", bufs=1) as singles,
            tc.tile_pool(name="hpool", bufs=3) as hpool,
            tc.tile_pool(name="dpool", bufs=3) as dpool,
            tc.tile_pool(name="work", bufs=3) as work,
            tc.tile_pool(name="psg", bufs=1, space="PSUM") as psg,
        ):
            # ---- load constants -------------------------------------------
            lat = [singles.tile([128, BS], f16, tag=f"lat{j}", name=f"lat{j}")
                   for j in range(nkl)]
            for j in range(nkl):
                nc.sync.dma_start(out=lat[j], in_=latT_d[128 * j:128 * (j + 1), :])
            wd = [singles.tile([128, H], f16, tag=f"wd{j}", name=f"wd{j}")
                  for j in range(nkl)]
            for j in range(nkl):
                nc.sync.dma_start(out=wd[j], in_=wd_d[128 * j:128 * (j + 1), :])
            w = [singles.tile([128, H3], f16, tag=f"w{j}", name=f"w{j}")
                 for j in range(nkl)]
            for j in range(nkl):
                nc.sync.dma_start(out=w[j], in_=w_d[128 * j:128 * (j + 1), :])
            u = [singles.tile([128, H3], f16, tag=f"u{k}", name=f"u{k}")
                 for k in range(nk)]
            for k in range(nk):
                nc.sync.dma_start(out=u[k], in_=u_d[128 * k:128 * (k + 1), :])
            bx = singles.tile([1, H3], f16, tag="bx")
            nc.gpsimd.dma_start(out=bx, in_=bx_d[:, :])
            b1h = singles.tile([1, H], f16, tag="b1h")
            nc.gpsimd.dma_start(out=b1h, in_=b1h_d[:, :])
            bd = singles.tile([1, H], f16, tag="bd")
            nc.gpsimd.dma_start(out=bd, in_=bd_d[:, :])
            ones = singles.tile([1, 128], f16, tag="ones")
            nc.vector.memset(ones, 1.0)

            # persistent gate banks + 2 prologue scratch banks
            ps_z = psg.tile([128, H], f32, tag="ps_z")
            ps_r = psg.tile([128, H], f32, tag="ps_r")
            ps_h = psg.tile([128, H], f32, tag="ps_h")
            ps_a = psg.tile([128, H], f32, tag="ps_a")
            ps_b = psg.tile([128, H], f32, tag="ps_b")
            gate_ps = {GZ: ps_z, GR: ps_r, GH: ps_h}

            def cs(m):
                return slice(128 * m, 128 * (m + 1))

            # ---- prologue --------------------------------------------------
            # PSUM start_tensor_calc lazily zeroes the WHOLE 2KB zero region
            # (= the full bank row), so each bank gets exactly ONE start=True
            # (its first matmul); every other matmul accumulates.  The first
            # write to each not-yet-touched region then replaces (pending
            # zero), later writes accumulate -- which is what we want.
            # h0 = tanh((latent @ Wd)^T + bd), stacked
            for m in range(nm):
                for j in range(nkl):
                    nc.tensor.matmul(ps_a[:, cs(m)], wd[j][:, cs(m)], lat[j],
                                     start=(j == 0 and m == 0), stop=False,
                                     skip_group_check=True)
                nc.tensor.matmul(ps_a[:, cs(m)], bd[:, cs(m)], ones,
                                 start=False, stop=True, skip_group_check=True)
            h = hpool.tile([128, H], f16, tag="h")
            nc.scalar.activation(h, ps_a, AF.Tanh)

            # xh = (latent @ W_h)^T + bx_h, stacked, fp16 in SBUF
            for m in range(nm):
                for j in range(nkl):
                    nc.tensor.matmul(ps_b[:, cs(m)], w[j][:, GH * H + 128 * m:
                                                          GH * H + 128 * (m + 1)],
                                     lat[j], start=(j == 0 and m == 0),
                                     stop=False, skip_group_check=True)
                nc.tensor.matmul(ps_b[:, cs(m)],
                                 bx[:, GH * H + 128 * m:GH * H + 128 * (m + 1)],
                                 ones, start=False, stop=True,
                                 skip_group_check=True)
            xh = singles.tile([128, H], f16, tag="xh")
            nc.scalar.copy(xh, ps_b)

            # gate banks: x-projection + bias + h0 @ U_g
            for g in (GZ, GR, GH):
                ps = gate_ps[g]
                first = [True]

                def mm(dst, lhsT, rhs, stop=False):
                    nc.tensor.matmul(dst, lhsT, rhs, start=first[0], stop=stop,
                                     skip_group_check=True)
                    first[0] = False

                for m in range(nm):
                    if g == GH:
                        # h gate: recurrent bias only (xh is separate)
                        mm(ps[:, cs(m)], b1h[:, cs(m)], ones)
                    else:
                        for j in range(nkl):
                            mm(ps[:, cs(m)],
                               w[j][:, g * H + 128 * m:g * H + 128 * (m + 1)],
                               lat[j])
                        mm(ps[:, cs(m)],
                           bx[:, g * H + 128 * m:g * H + 128 * (m + 1)],
                           ones)
                    for k in range(nk):
                        mm(ps[:, cs(m)],
                           u[k][:, g * H + 128 * m:g * H + 128 * (m + 1)],
                           h[:, cs(k)], stop=(k == nk - 1))

            # ---- steady-state T loop --------------------------------------
            dl_prev = None
            for t in range(T):
                # matmul round t: ps_g += dl_{t} @ U_g, one group per dl
                # chunk, fired as soon as that chunk exists.  Gate order:
                # r first (spine head), then z (zb needed mid-tail), then h.
                if dl_prev is not None:
                    mmb = cfg.get("mm_blocks")
                    if mmb == "GM":
                        mmb = MM_GM
                    elif mmb == "SMART":
                        mmb = MM_SMART
                    elif mmb == "GMT":
                        mmb = MM_GM_TAPER
                    if mmb:
                        blocks = [(GMAP[gc], ms, ks)
                                  for gc, ms, ks in mmb]
                    else:
                        blocks = []
                        for (lo, hi) in spans:
                            ks = tuple(range(lo // 128, hi // 128))
                            for g in gate_seq:
                                blocks.append((g, tuple(range(nm)), ks))
                    for g, ms, ks in blocks:
                        ps = gate_ps[g]
                        for m in ms:
                            for k in ks:
                                _lab(nc.tensor.matmul(
                                    ps[:, cs(m)],
                                    u[k][:, g * H + 128 * m:
                                         g * H + 128 * (m + 1)],
                                    dl_prev[:, cs(k)],
                                    start=False, stop=(k == nk - 1),
                                    skip_group_check=True),
                                    f"mm.{'zrh'[g]}.m{m}.k{k}")

                # tail t
                r = work.tile([128, H], f16, tag="r")
                zb = work.tile([128, H], f16, tag="zb")
                t1 = work.tile([128, H], f16, tag="t1")
                t2 = work.tile([128, H], f16, tag="t2")
                hh = work.tile([128, H], f16, tag="hh")
                e = work.tile([128, H], f16, tag="e")
                dl = dpool.tile([128, H], f16, tag="dl", name=f"dl{t % 3}")
                hn = hpool.tile([128, H], f16, tag="h", name=f"h{t % 3}")

                for ci, (lo, hi) in enumerate(sr_spans):
                    sp = slice(lo, hi)
                    _lab(nc.scalar.activation(r[:, sp], ps_r[:, sp],
                                              AF.Sigmoid), f"sr.c{ci}")
                for ci, (lo, hi) in enumerate(zspans):
                    sp = slice(lo, hi)
                    _lab(nc.scalar.activation(zb[:, sp], ps_z[:, sp],
                                              AF.Sigmoid, scale=-1.0),
                         f"zb.c{ci}")
                for ci, (lo, hi) in enumerate(t_spans):
                    sp = slice(lo, hi)
                    _lab(nc.vector.tensor_tensor(out=t1[:, sp], in0=r[:, sp],
                                                 in1=ps_h[:, sp], op=OP.mult),
                         f"t1.c{ci}")
                    _lab(nc.vector.tensor_tensor(out=t2[:, sp], in0=t1[:, sp],
                                                 in1=xh[:, sp], op=OP.add),
                         f"t2.c{ci}")
                for ci, (lo, hi) in enumerate(th_spans):
                    sp = slice(lo, hi)
                    _lab(nc.scalar.activation(hh[:, sp], t2[:, sp], AF.Tanh),
                         f"th.c{ci}")
                for ci, (lo, hi) in enumerate(e_spans):
                    sp = slice(lo, hi)
                    _lab(nc.vector.tensor_tensor(out=e[:, sp], in0=hh[:, sp],
                                                 in1=h[:, sp],
                                                 op=OP.subtract), f"e.c{ci}")
                for ci, (lo, hi) in enumerate(dl_spans):
                    sp = slice(lo, hi)
                    _lab(nc.vector.tensor_tensor(out=dl[:, sp], in0=zb[:, sp],
                                                 in1=e[:, sp], op=OP.mult),
                         f"dl.c{ci}")
                for ci, (lo, hi) in enumerate(spans):
                    sp = slice(lo, hi)
                    _lab(nc.gpsimd.tensor_tensor(out=hn[:, sp], in0=h[:, sp],
                                                 in1=dl[:, sp], op=OP.add),
                         f"hn.c{ci}")

                _lab(nc.sync.dma_start(out=out_d[t], in_=hn), "dma.out")
                dl_prev = dl
                h = hn

    nc.compile()
    return nc



DUAL_CFG = dict(dma_alt=False, split_emit=False, sr_halves=False,
                widths=WIDTHS, rz_merge=False)


def _build_dual(T, cfg=None):
    dcfg = dict(DUAL_CFG)
    if cfg:
        dcfg.update(cfg)
    widths = list(dcfg["widths"])
    NCH = len(widths)
    offs = [sum(widths[:i]) for i in range(NCH)]          # batch col offsets
    soffs = [4 * o for o in offs]                          # stacked col offsets
    """Two independent 64-batch chains per core; each chain's spine is
    hidden behind the other's engine work.  Per-chain tiles are [128, 256]
    stacked as (m, b64): S[p, 64m+b] = X[b, 128m+p]."""
    import concourse.bass as bass
    import concourse.mybir as mybir
    import concourse.tile as tile
    from concourse import bacc

    f32 = mybir.dt.float32
    f16 = mybir.dt.float16
    AF = mybir.ActivationFunctionType
    OP = mybir.AluOpType

    nc = bacc.Bacc(None, target_bir_lowering=False, debug=False)

    latT_d = nc.dram_tensor("latT", [LD, BS], f16, kind="ExternalInput")
    wd_d = nc.dram_tensor("wd", [LD, H], f16, kind="ExternalInput")
    w_d = nc.dram_tensor("w", [LD, H3], f16, kind="ExternalInput")
    u_d = nc.dram_tensor("u", [H, H3], f16, kind="ExternalInput")
    bx_d = nc.dram_tensor("bx", [1, H3], f16, kind="ExternalInput")
    b1h_d = nc.dram_tensor("b1h", [1, H], f16, kind="ExternalInput")
    bd_d = nc.dram_tensor("bd", [1, H], f16, kind="ExternalInput")
    # out[t, p, 256*c + 64*m + b] = h_{t+1}[64c + b, 128*m + p]
    out_d = nc.dram_tensor("out", [T, 128, H], f16, kind="ExternalOutput")

    nk = H // 128
    nm = H // 128
    nkl = LD // 128
    GZ, GR, GH = 0, 1, 2

    def gcols(g, m):
        return slice(g * H + 128 * m, g * H + 128 * (m + 1))

    with tile.TileContext(nc) as tc:
        with (
            tc.tile_pool(name="singles", bufs=1) as singles,
            tc.tile_pool(name="hpool", bufs=3) as hpool,
            tc.tile_pool(name="dpool", bufs=3) as dpool,
            tc.tile_pool(name="work", bufs=3) as work,
            tc.tile_pool(name="psg", bufs=1, space="PSUM") as psg,
        ):
            lat = [singles.tile([128, BS], f16, tag=f"lat{j}", name=f"lat{j}")
                   for j in range(nkl)]
            for j in range(nkl):
                nc.sync.dma_start(out=lat[j], in_=latT_d[128 * j:128 * (j + 1), :])
            wd = [singles.tile([128, H], f16, tag=f"wd{j}", name=f"wd{j}")
                  for j in range(nkl)]
            for j in range(nkl):
                nc.sync.dma_start(out=wd[j], in_=wd_d[128 * j:128 * (j + 1), :])
            w = [singles.tile([128, H3], f16, tag=f"w{j}", name=f"w{j}")
                 for j in range(nkl)]
            for j in range(nkl):
                nc.sync.dma_start(out=w[j], in_=w_d[128 * j:128 * (j + 1), :])
            u = [singles.tile([128, H3], f16, tag=f"u{k}", name=f"u{k}")
                 for k in range(nk)]
            for k in range(nk):
                nc.sync.dma_start(out=u[k], in_=u_d[128 * k:128 * (k + 1), :])
            bx = singles.tile([1, H3], f16, tag="bx")
            nc.gpsimd.dma_start(out=bx, in_=bx_d[:, :])
            b1h = singles.tile([1, H], f16, tag="b1h")
            nc.gpsimd.dma_start(out=b1h, in_=b1h_d[:, :])
            bd = singles.tile([1, H], f16, tag="bd")
            nc.gpsimd.dma_start(out=bd, in_=bd_d[:, :])
            ones = singles.tile([1, 128], f16, tag="ones")
            nc.vector.memset(ones, 1.0)

            rz_merge = dcfg.get("rz_merge", False)
            if rz_merge:
                # per-chain bank holds [r-gate || z-gate] contiguously so ONE
                # sigmoid pass covers both; h-gate shared bank for all chains.
                names = [f"prz{c}" for c in range(NCH)] + ["ph", "psa", "psb"]
                banks = {n: psg.tile([128, H], f32, tag=n, name=n)
                         for n in names}
                gate_ps = {c: {GZ: banks[f"prz{c}"], GR: banks[f"prz{c}"],
                               GH: banks["ph"]} for c in range(NCH)}
                gate_bn = {}

                def bnof(c, g):
                    return "ph" if g == GH else f"prz{c}"

                def bsl(c, m, g=None):
                    W2 = widths[c]
                    if g == GH or g is None:
                        base = soffs[c]
                    elif g == GR:
                        base = 0
                    else:  # GZ
                        base = 4 * W2
                    return slice(base + W2 * m, base + W2 * (m + 1))
            else:
                # one bank per gate, shared by all chains via disjoint regions
                names = ["pz", "pr", "ph", "psa", "psb"]
                banks = {n: psg.tile([128, H], f32, tag=n, name=n)
                         for n in names}
                gate_ps = {c: {GZ: banks["pz"], GR: banks["pr"],
                               GH: banks["ph"]} for c in range(NCH)}

                def bnof(c, g):
                    return {GZ: "pz", GR: "pr", GH: "ph"}[g]

                def bsl(c, m, g=None):
                    W2 = widths[c]
                    return slice(soffs[c] + W2 * m, soffs[c] + W2 * (m + 1))

            ps_first = {n: [True] for n in names}

            def mm(bank_name, dst, lhsT, rhs, stop=False):
                f = ps_first[bank_name]
                nc.tensor.matmul(dst, lhsT, rhs, start=f[0], stop=stop,
                                 skip_group_check=True)
                f[0] = False

            # ---- prologue (all chains) --------------------------------
            hs = []
            xhs = []
            for c in range(NCH):
                W2 = widths[c]
                bs = slice(offs[c], offs[c] + W2)  # batch cols in lat tiles
                po = soffs[c]  # offset into shared prologue banks
                ps_a, ps_b = banks["psa"], banks["psb"]
                for m in range(nm):
                    for j in range(nkl):
                        mm("psa", ps_a[:, po + W2 * m: po + W2 * (m + 1)],
                           wd[j][:, 128 * m:128 * (m + 1)], lat[j][:, bs])
                    mm("psa", ps_a[:, po + W2 * m: po + W2 * (m + 1)],
                       bd[:, 128 * m:128 * (m + 1)], ones[:, 0:W2],
                       stop=(m == nm - 1))
                h = hpool.tile([128, 4 * W2], f16, tag=f"h{c}",
                               name=f"h_{c}_init")
                nc.scalar.activation(h, ps_a[:, po:po + 4 * W2], AF.Tanh)

                for m in range(nm):
                    for j in range(nkl):
                        mm("psb", ps_b[:, po + W2 * m: po + W2 * (m + 1)],
                           w[j][:, gcols(GH, m)], lat[j][:, bs])
                    mm("psb", ps_b[:, po + W2 * m: po + W2 * (m + 1)],
                       bx[:, gcols(GH, m)], ones[:, 0:W2], stop=(m == nm - 1))
                xh = singles.tile([128, 4 * W2], f16, tag=f"xh{c}")
                nc.scalar.copy(xh, ps_b[:, po:po + 4 * W2])
                hs.append(h)
                xhs.append(xh)

                for g in (GZ, GR, GH):
                    ps = gate_ps[c][g]
                    bn = bnof(c, g)
                    for m in range(nm):
                        if g == GH:
                            mm(bn, ps[:, bsl(c, m, g)],
                               b1h[:, 128 * m:128 * (m + 1)], ones[:, 0:W2])
                        else:
                            for j in range(nkl):
                                mm(bn, ps[:, bsl(c, m, g)],
                                   w[j][:, gcols(g, m)], lat[j][:, bs])
                            mm(bn, ps[:, bsl(c, m, g)], bx[:, gcols(g, m)],
                               ones[:, 0:W2])
                        for k in range(nk):
                            mm(bn, ps[:, bsl(c, m, g)], u[k][:, gcols(g, m)],
                               h[:, slice(W2 * k, W2 * (k + 1))],
                               stop=(k == nk - 1))

            # ---- T loop: software-pipelined chain interleave ----------
            # Emission order (per t):  s1(A,t)  s2(B,t-1)  s1(B,t)  s2(A,t)
            # so each chain's tanh is immediately followed (in per-engine
            # program order) by its own next-step sigmoid, matching the
            # half-cycle phase offset between the chains.
            dls = [None] * NCH
            pend = [None] * NCH

            def s1(c, t):
                W2 = widths[c]
                # matmul round (consumes dls[c]) + sigmoids + t1/t2
                gorder = (GR, GZ, GH) if rz_merge else (GR, GH, GZ)
                if dls[c] is not None:
                    for g in gorder:
                        ps = gate_ps[c][g]
                        for m in range(nm):
                            for k in range(nk):
                                _lab(nc.tensor.matmul(
                                    ps[:, bsl(c, m, g)], u[k][:, gcols(g, m)],
                                    dls[c][:, W2 * k:W2 * (k + 1)],
                                    start=False, stop=(k == nk - 1),
                                    skip_group_check=True),
                                    f"mm{c}.{'zrh'[g]}.m{m}.k{k}")
                ps_h = gate_ps[c][GH]
                hsl = slice(soffs[c], soffs[c] + 4 * W2)
                t1 = work.tile([128, 4 * W2], f16, tag=f"t1{c}")
                t2 = work.tile([128, 4 * W2], f16, tag=f"t2{c}")
                if rz_merge:
                    prz = banks[f"prz{c}"]
                    rz = work.tile([128, 8 * W2], f16, tag=f"rz{c}")
                    _lab(nc.scalar.activation(rz, prz[:, 0:8 * W2],
                                              AF.Sigmoid), f"srz{c}")
                    r = rz[:, 0:4 * W2]
                    zb = work.tile([128, 4 * W2], f16, tag=f"zb{c}")
                    _lab(nc.vector.tensor_scalar(
                        out=zb, in0=rz[:, 4 * W2:8 * W2], scalar1=-1.0,
                        scalar2=1.0, op0=OP.mult, op1=OP.add), f"zbv{c}")
                else:
                    ps_z, ps_r = gate_ps[c][GZ], gate_ps[c][GR]
                    r = work.tile([128, 4 * W2], f16, tag=f"r{c}")
                    zb = work.tile([128, 4 * W2], f16, tag=f"zb{c}")
                    _lab(nc.scalar.activation(r, ps_r[:, bsl(c, 0, GR).start:
                                                    bsl(c, 3, GR).stop],
                                              AF.Sigmoid), f"sr{c}")
                    _lab(nc.scalar.activation(zb, ps_z[:, bsl(c, 0, GZ).start:
                                                     bsl(c, 3, GZ).stop],
                                              AF.Sigmoid, scale=-1.0),
                         f"zb{c}")
                _lab(nc.vector.tensor_tensor(out=t1, in0=r,
                                             in1=ps_h[:, hsl],
                                             op=OP.mult), f"t1{c}")
                _lab(nc.vector.tensor_tensor(out=t2, in0=t1, in1=xhs[c],
                                             op=OP.add), f"t2{c}")
                return (t, zb, t2)

            def s2(c):
                W2 = widths[c]
                t, zb, t2 = pend[c]
                h = hs[c]
                hh = work.tile([128, 4 * W2], f16, tag=f"hh{c}")
                e = work.tile([128, 4 * W2], f16, tag=f"e{c}")
                dl = dpool.tile([128, 4 * W2], f16, tag=f"dl{c}",
                                name=f"dl{c}_{t % 3}")
                hn = hpool.tile([128, 4 * W2], f16, tag=f"h{c}",
                                name=f"h{c}_{t % 3}")
                _lab(nc.scalar.activation(hh, t2, AF.Tanh), f"th{c}")
                _lab(nc.vector.tensor_tensor(out=e, in0=hh, in1=h,
                                             op=OP.subtract), f"e{c}")
                _lab(nc.vector.tensor_tensor(out=dl, in0=zb, in1=e,
                                             op=OP.mult), f"dl{c}")
                _lab(nc.gpsimd.tensor_tensor(out=hn, in0=h, in1=dl,
                                             op=OP.add), f"hn{c}")
                dma_eng = nc.gpsimd if (dcfg["dma_alt"] and c % 2 == 1) \
                    else nc.sync
                _lab(dma_eng.dma_start(
                    out=out_d[t][:, soffs[c]:soffs[c] + 4 * W2],
                    in_=hn), f"dma{c}")
                dls[c] = dl
                hs[c] = hn

            for t in range(T):
                for c in range(NCH):
                    pend[c] = s1(c, t)
                    cprev = (c - 1) % NCH
                    if pend[cprev] is not None and (t > 0 or c > 0):
                        s2(cprev)
                        pend[cprev] = None
            for c in range(NCH):
                if pend[c] is not None:
                    s2(c)

    nc.compile()
    return nc


def kernel(latent, Wd, bd, W, U, b, T, _trace=False):
    from concourse.bass_utils import run_bass_kernel_spmd

    latent = np.asarray(latent, dtype=np.float32)
    Wd = np.asarray(Wd, dtype=np.float32)
    bd = np.asarray(bd, dtype=np.float32)
    W = np.asarray(W, dtype=np.float32)
    U = np.asarray(U, dtype=np.float32)
    b = np.asarray(b, dtype=np.float32)
    T = int(T)

    key = (T,)
    if key not in _BUILD_CACHE:
        _BUILD_CACHE[key] = _build_dual(T) if ARCH == "dual" else _build(T)
    nc = _BUILD_CACHE[key]

    bx = b[0].copy()
    bx[: 2 * H] += b[1][: 2 * H]
    bx16 = np.ascontiguousarray(bx.reshape(1, H3)).astype(np.float16)
    b1h16 = np.ascontiguousarray(b[1][2 * H:].reshape(1, H)).astype(np.float16)
    bd16 = np.ascontiguousarray(bd.reshape(1, H)).astype(np.float16)
    u16 = np.ascontiguousarray(U).astype(np.float16)
    w16 = np.ascontiguousarray(W).astype(np.float16)
    wd16 = np.ascontiguousarray(Wd).astype(np.float16)

    in_maps = []
    for c in range(NCORES):
        rows = slice(c * BS, (c + 1) * BS)
        in_maps.append({
            "latT": np.ascontiguousarray(latent[rows].T).astype(np.float16),
            "wd": wd16, "w": w16, "u": u16,
            "bx": bx16, "b1h": b1h16, "bd": bd16,
        })

    res = run_bass_kernel_spmd(nc, in_maps, core_ids=list(range(NCORES)),
                               trace=_trace)
    if _trace and res.exec_time_ns is not None:
        print(f"HW exec time: {res.exec_time_ns} ns")
        if res.instructions_and_trace is not None:
            print(f"trace: {res.instructions_and_trace[1]}")

    # device output is stacked-transposed fp16; reassemble per arch.
    outs = []
    for c in range(NCORES):
        arr = res.results[c]["out"]  # [T, 128, 512] fp16
        if ARCH == "dual":
            # chain cc: arr[t, p, 4*off + w*m + b] = h_{t+1}[off + b, 128m + p]
            parts = []
            off = 0
            for wch in WIDTHS:
                sub = arr[:, :, 4 * off:4 * (off + wch)]
                sub = sub.reshape(T, 128, H // 128, wch).transpose(3, 0, 2, 1)
                parts.append(sub.reshape(wch, T, H))
                off += wch
            outs.append(np.concatenate(parts, axis=0))
        else:
            # arr[t, p, 128*m + b] = h_{t+1}[b, 128*m + p]
            arr = arr.reshape(T, 128, H // 128, 128).transpose(3, 0, 2, 1)
            outs.append(arr.reshape(BS, T, H))
    return np.ascontiguousarray(np.concatenate(outs, axis=0)).astype(np.float32)
